# revision 9
# baseline (speedup 1.0000x reference)
"""8-core Trainium2 Bass kernel v3 for nn_Attention_89489938579587.

reference: qkv = x @ w_attn.T; split q,k,v per 16 heads (HD=128); RoPE
(interleaved pairs); non-causal SDPA; y @ w_proj.T.  B=4, T=2048, D=2048.

Sharding: core i -> (batch b=i//2, head-half hh=i%2).  Each core computes
QKV for its 8 heads over ALL 2048 tokens, RoPE, SDPA, and a PARTIAL output
projection (contraction over its 1024 head-dims) in f-major layout
[2048 f, 2048 t].  Host adds the two partials per batch and transposes.
Ideal 1/8 compute share (25.8 GMAC/core = 3072 N=512 matmuls); no
collectives.

v3 changes over v2 (778 us):
 - softmax denominator reduce + broadcast moved off the PE: gpsimd
   partition_all_reduce + DVE reciprocal + fused scalar_tensor_tensor
   normalize (removes 64 aux/bc matmuls = ~22 us PE busy + disruption).
 - output projection restarted tq-major and started EARLY: 8 units
   interleaved into head-7's SDPA (PE filler while the exp chain drains;
   kills the 2.5-2.9 us/qt stalls + HAM 4/8 oscillation seen in the v2
   trace tail).  w_proj fully SBUF-resident after x is freed at h3.
 - bootstrap: x DMA'd in 512-token blocks with weight DMAs front-loaded
   so the first matmul starts ~7 us (was 16.8); junk matmuls on a memset
   tile pre-warm the HAM clock gate; dummy exp preloads the act table.
 - producer psum->sbuf copies moved from scalar to vector so the scalar
   engine only runs exp (it was the binding engine in the tail).
"""

import numpy as np
from contextlib import ExitStack

import concourse.bass as bass
import concourse.tile as tile
from concourse import mybir
from concourse import bass_isa
from concourse import library_config
from concourse.bass import ts

import bass_rust
import ml_dtypes

# ---------------------------------------------------------------------------
# Toolchain workarounds (same as baseline): walrus rejects >1 sem wait per
# instruction; split extras onto same-engine nops; patch tile drain.
# ---------------------------------------------------------------------------


def _split_multi_waits(nc, max_waits=1):
    n = 0
    for fn in nc.m.functions:
        for blk in fn.blocks:
            insts = blk.instructions
            i = 0
            while i < len(insts):
                inst = insts[i]
                si = inst.sync_info
                waits = list(si.on_wait) if (si is not None and si.on_wait) else []
                if len(waits) > max_waits:
                    si.on_wait = waits[:max_waits]
                    extra = waits[max_waits:]
                    for j in range(0, len(extra), max_waits):
                        nop = mybir.InstNoOp(
                            name=nc.get_next_instruction_name(), ins=[], outs=[])
                        nop.engine = inst.engine
                        nop.sync_info = bass_rust.SyncInfo(
                            on_wait=extra[j:j + max_waits], on_update=[])
                        nc.register_instruction(nop, overwrite=True)
                        insts.insert(i, nop)
                        i += 1
                        n += 1
                i += 1
    return n


def _patched_drain_and_barrier(self, tick_clock, wait_clock):
    from concourse.vector_clock import ScopedClock
    nc = self.nc
    probe = nc.sync.nop()
    wait_clock.add_sem_waits(probe.ins, ScopedClock({None: tick_clock.global_clock}))
    si = probe.ins.sync_info
    waits = list(si.on_wait or []) if si is not None else []
    if len(waits) > 1:
        si.on_wait = [waits[0]]
        for w in waits[1:]:
            nop = nc.sync.nop()
            nsi = nop.ins.sync_info
            if nsi is None:
                nop.ins.sync_info = bass_rust.SyncInfo(on_wait=[w], on_update=[])
            else:
                nsi.on_wait = [w]
    nc.sync.drain()
    nc.all_engine_barrier()
    assert self.sems is not None
    popped = nc._tile_sem_poison_stack.pop()
    assert popped is self._sem_poison
    nc.clear_and_free_semaphores(list(self.sems.allocated().values()))
    nc.all_engine_barrier()


_patched = False


def _apply_patches():
    global _patched
    if not _patched:
        tile.TileContext._drain_and_barrier = _patched_drain_and_barrier
        _patched = True


# ---------------------------------------------------------------------------
# Problem constants
# ---------------------------------------------------------------------------
BF16 = mybir.dt.bfloat16
F32 = mybir.dt.float32
EXP = mybir.ActivationFunctionType.Exp
MULT = mybir.AluOpType.mult

B, T, D, H, HD = 4, 2048, 2048, 16, 128
CC = D // 128            # 16 contraction chunks
NH = 8                   # heads per core
KC = T // 128            # 16 key chunks
NB = 4                   # 512-token x blocks
SCALE = 1.0 / float(np.sqrt(HD))
N_CORES = 8

# out-proj units interleaved into head-7's SDPA: (qt, kc) -> fc of tq0
OP_SLOTS = {(2, 1): 0, (2, 5): 1, (2, 9): 2, (2, 13): 3,
            (3, 1): 4, (3, 5): 5, (3, 9): 6, (3, 13): 7}


def build_nc(n_cores=N_CORES):
    _apply_patches()
    nc = bass.Bass("TRN2", target_bir_lowering=False, debug=False,
                   num_devices=n_cores)
    xT = nc.dram_tensor("xT", [CC, 128, T], BF16, kind="ExternalInput").ap()
    wqs = nc.dram_tensor("wqs", [NH, 128, CC * 128], BF16, kind="ExternalInput").ap()
    wks = nc.dram_tensor("wks", [NH, 128, CC * 128], BF16, kind="ExternalInput").ap()
    # V weights: [group, cc-half, 128p, 8cc * (4h*128f)]
    wvs = nc.dram_tensor("wvs", [2, 2, 128, 8 * 512], BF16, kind="ExternalInput").ap()
    # out-proj: [128p(hd), fc, hc, 128f]
    wps = nc.dram_tensor("wps", [128, CC, NH, 128], BF16, kind="ExternalInput").ap()
    cs2 = nc.dram_tensor("cs2", [128, T], BF16, kind="ExternalInput").ap()
    sn2 = nc.dram_tensor("sn2", [128, T], BF16, kind="ExternalInput").ap()
    # f-major partial output [2048 f, 2048 t]
    out = nc.dram_tensor("out", [D, T], BF16, kind="ExternalOutput").ap()

    with tile.TileContext(nc) as tc, ExitStack() as octx:
        # gpsimd ucode library with partition_all_reduce (loads at t=0,
        # first use is ~60 us in)
        nc.gpsimd.load_library(library_config.attn)

        cs_pool = octx.enter_context(tc.tile_pool(name="cs", bufs=1))
        csk = cs_pool.tile([128, T], BF16, tag="csk")
        snk = cs_pool.tile([128, T], BF16, tag="snk")
        jw = cs_pool.tile([128, 512], BF16, tag="jw")
        dexp = cs_pool.tile([1, 16], BF16, tag="dexp")

        oT_pool = octx.enter_context(tc.tile_pool(name="oT", bufs=1))
        oT = oT_pool.tile([128, NH, T], BF16, tag="oT")

        with ExitStack() as p1:
            wqk_pool = p1.enter_context(tc.tile_pool(name="wqk", bufs=3))
            wv_pool = p1.enter_context(tc.tile_pool(name="wv", bufs=2))
            vg_pool = p1.enter_context(tc.tile_pool(name="vg", bufs=1))
            q_pool = p1.enter_context(tc.tile_pool(name="q", bufs=2))
            k_pool = p1.enter_context(tc.tile_pool(name="k", bufs=2))
            rp_pool = p1.enter_context(tc.tile_pool(name="rp", bufs=2))
            eT_pool = p1.enter_context(tc.tile_pool(name="eT", bufs=5))
            es_pool = p1.enter_context(tc.tile_pool(name="es", bufs=2))
            eb_pool = p1.enter_context(tc.tile_pool(name="eb", bufs=2))
            rb_pool = p1.enter_context(tc.tile_pool(name="rb", bufs=2))
            ob_pool = p1.enter_context(tc.tile_pool(name="ob", bufs=6))
            wpf_pool = p1.enter_context(tc.tile_pool(name="wpf", bufs=4))
            ps_qkv = p1.enter_context(tc.tile_pool(name="psqkv", bufs=2, space="PSUM"))
            ps_s = p1.enter_context(tc.tile_pool(name="pss", bufs=3, space="PSUM"))
            ps_o = p1.enter_context(tc.tile_pool(name="pso", bufs=1, space="PSUM"))
            ps_op = p1.enter_context(tc.tile_pool(name="psop", bufs=2, space="PSUM"))

            x_pool = p1.enter_context(tc.tile_pool(name="x", bufs=1))

            # x resident as 64 [128, 512] block tiles (fine-grained DMA so
            # early consumers start as soon as their block lands)
            xs2 = [[None] * NB for _ in range(CC)]

            # per-core state holders
            q_sb = [None] * NH
            k_sb = [None] * NH
            v_sb = [None] * 2   # per group
            v_w = [[None, None], [None, None]]
            w_hold = {}
            sf_hold = [None]
            wp_slabs = {}

            def load_wp_slab(fc):
                w_ = wpf_pool.tile([128, NH, 128], BF16, tag="wpf",
                                   name=f"wpf{fc}")
                nc.sync.dma_start(w_[:], wps[:, fc])
                wp_slabs[fc] = w_

            def load_vw(g, half):
                w_ = wv_pool.tile([128, 8 * 512], BF16, tag="wv")
                nc.sync.dma_start(w_[:], wvs[g, half])
                v_w[g][half] = w_

            def ensure_wqk(kind, h):
                if (kind, h) not in w_hold:
                    wsl = wqk_pool.tile([128, CC, 128], BF16, tag="wqk")
                    nc.sync.dma_start(wsl[:], (wqs if kind == "q" else wks)[h])
                    w_hold[(kind, h)] = wsl
                return w_hold[(kind, h)]

            def emit_proj_half(kind, h, tpair, half):
                """One 16-matmul unit: 512 tokens of a Q/K projection.  After
                the second half of a token-pair, the RoPE chain is emitted."""
                wsl = ensure_wqk(kind, h)
                toff = tpair * 1024
                if half == 0:
                    sf_hold[0] = rp_pool.tile([128, 1024], BF16, tag="sf",
                                              name=f"sf_{kind}{h}_{tpair}")
                sf = sf_hold[0]
                ps = ps_qkv.tile([128, 512], F32, tag="psqkv")
                xb = tpair * 2 + half
                for cc in range(CC):
                    nc.tensor.matmul(
                        ps[:], wsl[:, cc, :], xs2[cc][xb][:],
                        start=(cc == 0), stop=(cc == CC - 1))
                nc.vector.tensor_copy(sf[:, ts(half, 512)], ps[:])
                if half == 0:
                    return
                sw = rp_pool.tile([128, 1024], BF16, tag="sw")
                nc.sync.dma_start(sw[0:64, :], sf[64:128, :])
                nc.sync.dma_start(sw[64:128, :], sf[0:64, :])
                nc.vector.tensor_mul(sf[:], sf[:], csk[:, toff:toff + 1024])
                nc.vector.tensor_mul(sw[:], sw[:], snk[:, toff:toff + 1024])
                dst = q_sb[h] if kind == "q" else k_sb[h]
                nc.vector.tensor_add(dst[:, toff:toff + 1024], sf[:], sw[:])

            def producer_units(hn):
                """Generator of head-hn QKV producer units (8 per head)."""
                q_sb[hn] = q_pool.tile([128, T], BF16, tag="q", name=f"qh{hn}")
                k_sb[hn] = k_pool.tile([128, T], BF16, tag="k", name=f"kh{hn}")
                for kind in ("q", "k"):
                    for tpair in range(2):
                        for half in range(2):
                            yield (kind, hn, tpair, half)

            def emit_v_chunk(g, tch_pair):
                """V for head-group g, two token chunks (2*128 tokens)."""
                for u in range(2):
                    tch = tch_pair * 2 + u
                    ps = ps_qkv.tile([128, 512], F32, tag="psqkv")
                    for cc in range(CC):
                        wv_ap = v_w[g][cc // 8][:, (cc % 8) * 512:(cc % 8 + 1) * 512]
                        nc.tensor.matmul(
                            ps[:], xs2[cc][tch // 4][:, ts(tch % 4, 128)], wv_ap,
                            start=(cc == 0), stop=(cc == CC - 1))
                    if tch % 2 == 0:
                        nc.scalar.copy(v_sb[g][:, tch, :], ps[:])
                    else:
                        nc.vector.tensor_copy(v_sb[g][:, tch, :], ps[:])

            op_state = {"alt": 0}

            def emit_op_unit(fc, tq, eng):
                """Partial out-proj: one [128f x 512t] tile, contraction over
                this core's 8 heads."""
                wp = wp_slabs[fc]
                ps = ps_op.tile([128, 512], F32, tag="psop")
                for hc in range(NH):
                    nc.tensor.matmul(ps[:], wp[:, hc, :],
                                     oT[:, hc, ts(tq, 512)],
                                     start=(hc == 0), stop=(hc == NH - 1))
                oe = ob_pool.tile([128, 512], BF16, tag="ob")
                if eng == "v":
                    nc.vector.tensor_copy(oe[:], ps[:])
                else:
                    nc.scalar.copy(oe[:], ps[:])
                nc.sync.dma_start(out[ts(fc, 128), ts(tq, 512)], oe[:])

            def emit_op_unit_alt(fc, tq):
                op_state["alt"] ^= 1
                emit_op_unit(fc, tq, "s" if op_state["alt"] else "v")

            # ---------------- bootstrap ----------------
            # HAM warm-up: junk matmuls on a memset tile while x DMA lands.
            nc.vector.memset(jw[:], 0.0)
            jp = ps_op.tile([128, 512], F32, tag="psop", name="junk")
            for i in range(14):
                nc.tensor.matmul(jp[:], jw[:, 0:128], jw[:],
                                 start=True, stop=True)

            # DMA order = sync-queue order: everything the first ~60 us of
            # PE work needs, in consumption order.
            load_vw(0, 0)
            ensure_wqk("q", 0)
            ensure_wqk("k", 0)
            for cc in range(CC):
                for b_ in range(NB):
                    t_ = x_pool.tile([128, 512], BF16, tag=f"x{cc}_{b_}",
                                     name=f"x{cc}_{b_}")
                    xs2[cc][b_] = t_
            for cc in range(CC):
                nc.sync.dma_start(xs2[cc][0][:], xT[cc][:, 0:512])
            nc.sync.dma_start(csk[:], cs2[:])
            nc.sync.dma_start(snk[:], sn2[:])
            # dummy exp right after csk arrives: walrus puts the ~2.7 us act
            # table load here, off the critical path
            nc.scalar.activation(dexp[:], csk[0:1, 0:16], EXP, scale=1.0)
            load_vw(0, 1)
            for b_ in range(1, NB):
                for cc in range(CC):
                    nc.sync.dma_start(xs2[cc][b_][:], xT[cc][:, ts(b_, 512)])
            load_vw(1, 0)
            load_vw(1, 1)

            v_sb[0] = vg_pool.tile([128, KC, 512], BF16, tag="vg", name="vg0")
            for tp in range(8):
                emit_v_chunk(0, tp)
            for unit in producer_units(0):
                emit_proj_half(*unit)

            # ---------------- main loop: SDPA per head ----------------
            for h in range(NH):
                g, j = h // 4, h % 4
                prod = producer_units(h + 1) if h + 1 < NH else iter(())
                for qt in range(4):
                    qsl = q_sb[h][:, ts(qt, 512)]
                    esum = es_pool.tile([128, 512], BF16, tag="es")
                    o_ps = ps_o.tile([128, 512], F32, tag="pso")
                    eTs = [None] * KC

                    def pv(kc):
                        nc.tensor.matmul(
                            o_ps[:], v_sb[g][:, kc, ts(j, 128)], eTs[kc][:],
                            start=(kc == 0), stop=(kc == KC - 1))

                    for kc in range(KC):
                        s_ps = ps_s.tile([128, 512], F32, tag="pss")
                        nc.tensor.matmul(s_ps[:], k_sb[h][:, ts(kc, 128)], qsl,
                                         start=True, stop=True)
                        eT = eT_pool.tile([128, 512], BF16, tag="eT")
                        nc.scalar.activation(eT[:], s_ps[:], EXP, scale=SCALE)
                        eTs[kc] = eT
                        # bf16 chunk-sum (magnitude ~30; the 2048-wide key
                        # reduction happens exactly in f32 on gpsimd below)
                        with nc.allow_low_precision(reason="bf16 chunk sum"):
                            if kc == 0:
                                nc.vector.tensor_copy(esum[:], eT[:])
                            else:
                                nc.vector.tensor_add(esum[:], esum[:], eT[:])
                        if kc >= 2:
                            pv(kc - 2)
                        if h < NH - 1 and kc in (5, 11):
                            unit = next(prod, None)
                            if unit is not None:
                                emit_proj_half(*unit)
                        if h == NH - 1:
                            if qt == 0 and kc == 1:
                                load_wp_slab(0)
                            elif qt == 0 and kc == 5:
                                load_wp_slab(1)
                            elif (qt, kc) in OP_SLOTS:
                                fc_ = OP_SLOTS[(qt, kc)]
                                emit_op_unit(fc_, 0, "v")
                                if fc_ + 2 < 10:
                                    load_wp_slab(fc_ + 2)
                    pv(KC - 2)
                    pv(KC - 1)
                    # softmax denominator: f32 partition all-reduce on gpsimd
                    # (every partition gets the 512 per-token sums), DVE
                    # reciprocal, then one fused (o * recip) normalize from
                    # PSUM into the f-major attention output staging tile.
                    eb = eb_pool.tile([128, 512], F32, tag="eb")
                    nc.gpsimd.partition_all_reduce(
                        eb[:], esum[:], 128, bass_isa.ReduceOp.add)
                    rb = rb_pool.tile([128, 512], BF16, tag="rb")
                    with nc.allow_low_precision(reason="bf16 softmax denom"):
                        nc.vector.reciprocal(rb[:], eb[:])
                        nc.vector.scalar_tensor_tensor(
                            oT[:, h, ts(qt, 512)], o_ps[:], 1.0, rb[:],
                            MULT, MULT)

                # V(g1) block between head 3 and head 4: v_sb[0] reads are
                # all emitted by now, so the single vg buffer can recycle.
                if h == 3:
                    v_sb[1] = vg_pool.tile([128, KC, 512], BF16, tag="vg",
                                           name="vg1")
                    for tp in range(8):
                        emit_v_chunk(1, tp)

            # ------------- rest of the output projection -------------
            # tq0 tail (slabs 10..15 + second-round 0,1 prefetched 2 ahead)
            for fc in range(8, CC):
                if fc + 2 < CC:
                    load_wp_slab(fc + 2)
                else:
                    load_wp_slab(fc + 2 - CC)
                emit_op_unit_alt(fc, 0)
            # tq 1..3 fc-major: one slab load per fc serves three units
            for fc in range(CC):
                for i, tq in enumerate((1, 2, 3)):
                    if i == 0 and fc + 2 < CC:
                        load_wp_slab(fc + 2)
                    emit_op_unit_alt(fc, tq)

    # populate .instr bytes for extended-inst InstISA subclasses
    # (partition_all_reduce) — raw Bass doesn't run this pass and the NEFF
    # compiler rejects empty .instr with "ISA wrong length"
    mybir.codegen_inst_isa_subclasses(nc)
    _split_multi_waits(nc)
    return nc


# ---------------------------------------------------------------------------
# host-side prep / assembly
# ---------------------------------------------------------------------------


def _to_bf16(a):
    return np.ascontiguousarray(a.astype(ml_dtypes.bfloat16))


def prep_inputs(x, w_attn, w_proj):
    x = np.asarray(x, dtype=np.float32)
    w_attn = np.asarray(w_attn, dtype=np.float32)
    w_proj = np.asarray(w_proj, dtype=np.float32)

    perm = np.concatenate([np.arange(0, HD, 2), np.arange(1, HD, 2)])

    inv = 1.0 / (10000.0 ** (np.arange(0, HD, 2, dtype=np.float64) / HD))
    fr = np.outer(np.arange(T, dtype=np.float64), inv)
    cos = np.cos(fr).T
    sin = np.sin(fr).T
    cs2 = _to_bf16(np.concatenate([cos, cos], 0))
    sn2 = _to_bf16(np.concatenate([-sin, sin], 0))

    # per head-half weight slabs (shared across batches)
    half_slabs = []
    for hh in range(2):
        heads = range(hh * NH, (hh + 1) * NH)
        # wq/wk: [NH, 128p(c within cc), CC*128f] with rope perm on f
        wq_sl = np.empty((NH, 128, CC * 128), dtype=np.float32)
        wk_sl = np.empty((NH, 128, CC * 128), dtype=np.float32)
        for jj, h in enumerate(heads):
            wq_h = w_attn[h * HD:(h + 1) * HD][perm, :]        # [128f, 2048c]
            wk_h = w_attn[D + h * HD:D + (h + 1) * HD][perm, :]
            # slab[p, cc, f] = w[f, cc*128+p]
            wq_sl[jj] = wq_h.T.reshape(CC, 128, 128).transpose(1, 0, 2).reshape(128, -1)
            wk_sl[jj] = wk_h.T.reshape(CC, 128, 128).transpose(1, 0, 2).reshape(128, -1)
        # wv: [2 groups, 2 halves, 128p, 8cc*(4h*128)]
        wv_sl = np.empty((2, 2, 128, 8 * 512), dtype=np.float32)
        for g in range(2):
            hv = w_attn[2 * D + (hh * NH + g * 4) * HD:
                        2 * D + (hh * NH + (g + 1) * 4) * HD]  # [512f, 2048c]
            # [cc, p, f] -> [half, 128p, 8cc, 512f]
            arr = hv.T.reshape(CC, 128, 512)
            for half in range(2):
                wv_sl[g, half] = (arr[half * 8:(half + 1) * 8]
                                  .transpose(1, 0, 2).reshape(128, -1))
        # wp: [128p(hd within hc), fc, hc, 128f]
        #   value = w_proj[fc*128+f, hh*1024 + hc*128 + p]
        wp_cols = w_proj[:, hh * NH * HD:(hh + 1) * NH * HD]  # [2048f, 1024hd]
        wp_sl = (wp_cols.T.reshape(NH, 128, CC, 128)
                 .transpose(1, 2, 0, 3))                       # [128p, fc, hc, f]
        half_slabs.append((_to_bf16(wq_sl), _to_bf16(wk_sl), _to_bf16(wv_sl),
                           _to_bf16(np.ascontiguousarray(wp_sl))))

    xTs = []
    for b in range(B):
        xT = x[b].T.reshape(CC, 128, T)
        xTs.append(_to_bf16(xT))

    in_maps = []
    for i in range(N_CORES):
        b, hh = i // 2, i % 2
        wq_sl, wk_sl, wv_sl, wp_sl = half_slabs[hh]
        in_maps.append({
            "xT": xTs[b],
            "wqs": wq_sl, "wks": wk_sl, "wvs": wv_sl, "wps": wp_sl,
            "cs2": cs2, "sn2": sn2,
        })
    return in_maps


def assemble(results):
    out = np.empty((B, T, D), dtype=np.float32)
    for b in range(B):
        p0 = results[2 * b]["out"].astype(np.float32)
        p1 = results[2 * b + 1]["out"].astype(np.float32)
        out[b] = (p0 + p1).T
    return out


_nc_cache = None


def _get_nc():
    global _nc_cache
    if _nc_cache is None:
        _nc_cache = build_nc()
    return _nc_cache


def kernel(x, w_attn, w_proj):
    from concourse.bass_utils import run_bass_kernel_spmd
    nc = _get_nc()
    in_maps = prep_inputs(x, w_attn, w_proj)
    res = run_bass_kernel_spmd(nc, in_maps, list(range(N_CORES)))
    return assemble(res.results)


def run_profiled(x, w_attn, w_proj, trace_cores=None):
    """Like kernel() but with NTFF profiling; returns BassKernelResults."""
    from concourse.bass_utils import run_bass_kernel_spmd
    import sys as _sys, types as _types
    try:
        import antenv
        if "antenv.axon_hooks" not in _sys.modules:
            mod = _types.ModuleType("antenv.axon_hooks")
            _h = [None]
            mod.set_axon_ntff_profile_hook = lambda h: _h.__setitem__(0, h)
            mod.get_axon_ntff_profile_hook = lambda: _h[0]
            _sys.modules["antenv.axon_hooks"] = mod
            antenv.axon_hooks = mod
            from trn_agent_boot.trn_boot import _ntff_profile_via_ctypes
            mod.set_axon_ntff_profile_hook(
                _ntff_profile_via_ctypes('/opt/axon/libaxon_pjrt.so'))
    except Exception as e:  # profiling is best-effort
        print("profile hook setup failed:", e)
    nc = _get_nc()
    in_maps = prep_inputs(x, w_attn, w_proj)
    return run_bass_kernel_spmd(
        nc, in_maps, list(range(N_CORES)), trace=True,
        trace_cores=trace_cores if trace_cores is not None else [0])


# revision 15
# speedup vs baseline: 1.0377x; 1.0377x over previous
"""8-core Trainium2 Bass kernel v3 for nn_Attention_89489938579587.

reference: qkv = x @ w_attn.T; split q,k,v per 16 heads (HD=128); RoPE
(interleaved pairs); non-causal SDPA; y @ w_proj.T.  B=4, T=2048, D=2048.

Sharding: core i -> (batch b=i//2, head-half hh=i%2).  Each core computes
QKV for its 8 heads over ALL 2048 tokens, RoPE, SDPA, and a PARTIAL output
projection (contraction over its 1024 head-dims) in f-major layout
[2048 f, 2048 t].  Host adds the two partials per batch and transposes.
Ideal 1/8 compute share (25.8 GMAC/core = 3072 N=512 matmuls); no
collectives.

v3 changes over v2 (778 us):
 - softmax denominator reduce + broadcast moved off the PE: gpsimd
   partition_all_reduce + DVE reciprocal + fused scalar_tensor_tensor
   normalize (removes 64 aux/bc matmuls = ~22 us PE busy + disruption).
 - output projection restarted tq-major and started EARLY: 8 units
   interleaved into head-7's SDPA (PE filler while the exp chain drains;
   kills the 2.5-2.9 us/qt stalls + HAM 4/8 oscillation seen in the v2
   trace tail).  w_proj fully SBUF-resident after x is freed at h3.
 - bootstrap: x DMA'd in 512-token blocks with weight DMAs front-loaded
   so the first matmul starts ~7 us (was 16.8); junk matmuls on a memset
   tile pre-warm the HAM clock gate; dummy exp preloads the act table.
 - producer psum->sbuf copies moved from scalar to vector so the scalar
   engine only runs exp (it was the binding engine in the tail).
"""

import numpy as np
from contextlib import ExitStack

import concourse.bass as bass
import concourse.tile as tile
from concourse import mybir
from concourse import bass_isa
from concourse import library_config
from concourse.bass import ts

import bass_rust
import ml_dtypes

# ---------------------------------------------------------------------------
# Toolchain workarounds (same as baseline): walrus rejects >1 sem wait per
# instruction; split extras onto same-engine nops; patch tile drain.
# ---------------------------------------------------------------------------


def _split_multi_waits(nc, max_waits=1):
    n = 0
    for fn in nc.m.functions:
        for blk in fn.blocks:
            insts = blk.instructions
            i = 0
            while i < len(insts):
                inst = insts[i]
                si = inst.sync_info
                waits = list(si.on_wait) if (si is not None and si.on_wait) else []
                if len(waits) > max_waits:
                    si.on_wait = waits[:max_waits]
                    extra = waits[max_waits:]
                    for j in range(0, len(extra), max_waits):
                        nop = mybir.InstNoOp(
                            name=nc.get_next_instruction_name(), ins=[], outs=[])
                        nop.engine = inst.engine
                        nop.sync_info = bass_rust.SyncInfo(
                            on_wait=extra[j:j + max_waits], on_update=[])
                        nc.register_instruction(nop, overwrite=True)
                        insts.insert(i, nop)
                        i += 1
                        n += 1
                i += 1
    return n


def _patched_drain_and_barrier(self, tick_clock, wait_clock):
    from concourse.vector_clock import ScopedClock
    nc = self.nc
    probe = nc.sync.nop()
    wait_clock.add_sem_waits(probe.ins, ScopedClock({None: tick_clock.global_clock}))
    si = probe.ins.sync_info
    waits = list(si.on_wait or []) if si is not None else []
    if len(waits) > 1:
        si.on_wait = [waits[0]]
        for w in waits[1:]:
            nop = nc.sync.nop()
            nsi = nop.ins.sync_info
            if nsi is None:
                nop.ins.sync_info = bass_rust.SyncInfo(on_wait=[w], on_update=[])
            else:
                nsi.on_wait = [w]
    nc.sync.drain()
    nc.all_engine_barrier()
    assert self.sems is not None
    popped = nc._tile_sem_poison_stack.pop()
    assert popped is self._sem_poison
    nc.clear_and_free_semaphores(list(self.sems.allocated().values()))
    nc.all_engine_barrier()


_patched = False


def _apply_patches():
    global _patched
    if not _patched:
        tile.TileContext._drain_and_barrier = _patched_drain_and_barrier
        _patched = True


# ---------------------------------------------------------------------------
# Problem constants
# ---------------------------------------------------------------------------
BF16 = mybir.dt.bfloat16
F32 = mybir.dt.float32
EXP = mybir.ActivationFunctionType.Exp
MULT = mybir.AluOpType.mult

B, T, D, H, HD = 4, 2048, 2048, 16, 128
CC = D // 128            # 16 contraction chunks
NH = 8                   # heads per core
KC = T // 128            # 16 key chunks
NB = 4                   # 512-token x blocks
SCALE = 1.0 / float(np.sqrt(HD))
N_CORES = 8

# out-proj units interleaved into head-7's SDPA: (qt, kc) -> fc of tq0
OP_SLOTS = {(2, 1): 0, (2, 5): 1, (2, 9): 2, (2, 13): 3,
            (3, 1): 4, (3, 5): 5, (3, 9): 6, (3, 13): 7}


def build_nc(n_cores=N_CORES):
    _apply_patches()
    nc = bass.Bass("TRN2", target_bir_lowering=False, debug=False,
                   num_devices=n_cores)
    xT = nc.dram_tensor("xT", [CC, 128, T], BF16, kind="ExternalInput").ap()
    wqs = nc.dram_tensor("wqs", [NH, 128, CC * 128], BF16, kind="ExternalInput").ap()
    wks = nc.dram_tensor("wks", [NH, 128, CC * 128], BF16, kind="ExternalInput").ap()
    # V weights: [group, cc-half, 128p, 8cc * (4h*128f)]
    wvs = nc.dram_tensor("wvs", [2, 2, 128, 8 * 512], BF16, kind="ExternalInput").ap()
    # out-proj: [128p(hd), fc, hc, 128f]
    wps = nc.dram_tensor("wps", [128, CC, NH, 128], BF16, kind="ExternalInput").ap()
    cs2 = nc.dram_tensor("cs2", [128, T], BF16, kind="ExternalInput").ap()
    sn2 = nc.dram_tensor("sn2", [128, T], BF16, kind="ExternalInput").ap()
    # f-major partial output [2048 f, 2048 t]
    out = nc.dram_tensor("out", [D, T], BF16, kind="ExternalOutput").ap()

    with tile.TileContext(nc) as tc, ExitStack() as octx:
        # gpsimd ucode library with partition_all_reduce (loads at t=0,
        # first use is ~60 us in)
        nc.gpsimd.load_library(library_config.attn)

        cs_pool = octx.enter_context(tc.tile_pool(name="cs", bufs=1))
        csk = cs_pool.tile([128, T], BF16, tag="csk")
        snk = cs_pool.tile([128, T], BF16, tag="snk")
        jw = cs_pool.tile([128, 512], BF16, tag="jw")
        dexp = cs_pool.tile([1, 16], BF16, tag="dexp")

        oT_pool = octx.enter_context(tc.tile_pool(name="oT", bufs=1))
        oT = oT_pool.tile([128, NH, T], BF16, tag="oT")

        with ExitStack() as p1:
            wqk_pool = p1.enter_context(tc.tile_pool(name="wqk", bufs=3))
            wv_pool = p1.enter_context(tc.tile_pool(name="wv", bufs=2))
            vg_pool = p1.enter_context(tc.tile_pool(name="vg", bufs=1))
            q_pool = p1.enter_context(tc.tile_pool(name="q", bufs=2))
            k_pool = p1.enter_context(tc.tile_pool(name="k", bufs=2))
            rp_pool = p1.enter_context(tc.tile_pool(name="rp", bufs=2))
            eT_pool = p1.enter_context(tc.tile_pool(name="eT", bufs=5))
            es_pool = p1.enter_context(tc.tile_pool(name="es", bufs=2))
            eb_pool = p1.enter_context(tc.tile_pool(name="eb", bufs=2))
            rb_pool = p1.enter_context(tc.tile_pool(name="rb", bufs=2))
            ob_pool = p1.enter_context(tc.tile_pool(name="ob", bufs=6))
            wpf_pool = p1.enter_context(tc.tile_pool(name="wpf", bufs=4))
            # 8 PSUM banks: qkv/out-proj units share one pool (producers are
            # done before the out-proj starts); ps_o double-buffered so the
            # slow gpsimd norm chain reads bank A while qt+1 accumulates in
            # bank B (chain gets ~28 us of slack, PE never waits on it)
            ps_qkv = p1.enter_context(tc.tile_pool(name="psqkv", bufs=2, space="PSUM"))
            ps_s = p1.enter_context(tc.tile_pool(name="pss", bufs=4, space="PSUM"))
            ps_o = p1.enter_context(tc.tile_pool(name="pso", bufs=2, space="PSUM"))

            x_pool = p1.enter_context(tc.tile_pool(name="x", bufs=1))

            # x resident as 64 [128, 512] block tiles (fine-grained DMA so
            # early consumers start as soon as their block lands)
            xs2 = [[None] * NB for _ in range(CC)]

            # per-core state holders
            q_sb = [None] * NH
            k_sb = [None] * NH
            v_sb = [None] * 2   # per group
            v_w = [[None, None], [None, None]]
            w_hold = {}
            sf_hold = {}
            wp_slabs = {}

            def load_wp_slab(fc):
                w_ = wpf_pool.tile([128, NH, 128], BF16, tag="wpf",
                                   name=f"wpf{fc}")
                nc.sync.dma_start(w_[:], wps[:, fc])
                wp_slabs[fc] = w_

            def load_vw(g, half):
                w_ = wv_pool.tile([128, 8 * 512], BF16, tag="wv")
                nc.sync.dma_start(w_[:], wvs[g, half])
                v_w[g][half] = w_

            def ensure_wqk(kind, h):
                if (kind, h) not in w_hold:
                    wsl = wqk_pool.tile([128, CC, 128], BF16, tag="wqk")
                    nc.sync.dma_start(wsl[:], (wqs if kind == "q" else wks)[h])
                    w_hold[(kind, h)] = wsl
                return w_hold[(kind, h)]

            def emit_proj_half(kind, h, tpair, half):
                """One 16-matmul unit: 512 tokens of a Q/K projection.  After
                the second half of a token-pair, the RoPE chain is emitted."""
                wsl = ensure_wqk(kind, h)
                toff = tpair * 1024
                if half == 0:
                    sf_hold[kind] = rp_pool.tile([128, 1024], BF16, tag="sf",
                                                 name=f"sf_{kind}{h}_{tpair}")
                sf = sf_hold[kind]
                ps = ps_qkv.tile([128, 512], F32, tag="psqkv")
                xb = tpair * 2 + half
                for cc in range(CC):
                    nc.tensor.matmul(
                        ps[:], wsl[:, cc, :], xs2[cc][xb][:],
                        start=(cc == 0), stop=(cc == CC - 1))
                nc.vector.tensor_copy(sf[:, ts(half, 512)], ps[:])
                if half == 0:
                    return
                sw = rp_pool.tile([128, 1024], BF16, tag="sw")
                nc.sync.dma_start(sw[0:64, :], sf[64:128, :])
                nc.sync.dma_start(sw[64:128, :], sf[0:64, :])
                nc.vector.tensor_mul(sf[:], sf[:], csk[:, toff:toff + 1024])
                nc.vector.tensor_mul(sw[:], sw[:], snk[:, toff:toff + 1024])
                dst = q_sb[h] if kind == "q" else k_sb[h]
                nc.vector.tensor_add(dst[:, toff:toff + 1024], sf[:], sw[:])

            def producer_units(hn):
                """Generator of head-hn QKV producer units (8 per head)."""
                q_sb[hn] = q_pool.tile([128, T], BF16, tag="q", name=f"qh{hn}")
                k_sb[hn] = k_pool.tile([128, T], BF16, tag="k", name=f"kh{hn}")
                for kind in ("q", "k"):
                    for tpair in range(2):
                        for half in range(2):
                            yield (kind, hn, tpair, half)

            def emit_v_chunk(g, tch_pair):
                """V for head-group g, two token chunks (2*128 tokens)."""
                for u in range(2):
                    tch = tch_pair * 2 + u
                    ps = ps_qkv.tile([128, 512], F32, tag="psqkv")
                    for cc in range(CC):
                        wv_ap = v_w[g][cc // 8][:, (cc % 8) * 512:(cc % 8 + 1) * 512]
                        nc.tensor.matmul(
                            ps[:], xs2[cc][tch // 4][:, ts(tch % 4, 128)], wv_ap,
                            start=(cc == 0), stop=(cc == CC - 1))
                    if tch % 2 == 0:
                        nc.scalar.copy(v_sb[g][:, tch, :], ps[:])
                    else:
                        nc.vector.tensor_copy(v_sb[g][:, tch, :], ps[:])

            op_state = {"alt": 0}

            def emit_op_unit(fc, tq, eng):
                """Partial out-proj: one [128f x 512t] tile, contraction over
                this core's 8 heads."""
                wp = wp_slabs[fc]
                ps = ps_qkv.tile([128, 512], F32, tag="psqkv")
                for hc in range(NH):
                    nc.tensor.matmul(ps[:], wp[:, hc, :],
                                     oT[:, hc, ts(tq, 512)],
                                     start=(hc == 0), stop=(hc == NH - 1))
                oe = ob_pool.tile([128, 512], BF16, tag="ob")
                if eng == "v":
                    nc.vector.tensor_copy(oe[:], ps[:])
                else:
                    nc.scalar.copy(oe[:], ps[:])
                nc.sync.dma_start(out[ts(fc, 128), ts(tq, 512)], oe[:])

            def emit_op_unit_alt(fc, tq):
                op_state["alt"] ^= 1
                emit_op_unit(fc, tq, "s" if op_state["alt"] else "v")

            # ---------------- bootstrap ----------------
            # HAM warm-up: junk matmuls on a memset tile while x DMA lands.
            nc.vector.memset(jw[:], 0.0)
            jp = ps_qkv.tile([128, 512], F32, tag="psqkv", name="junk")
            for i in range(14):
                nc.tensor.matmul(jp[:], jw[:, 0:128], jw[:],
                                 start=True, stop=True)

            # DMA order = sync-queue order: everything the first ~60 us of
            # PE work needs, in consumption order.
            load_vw(0, 0)
            for cc in range(CC):
                for b_ in range(NB):
                    t_ = x_pool.tile([128, 512], BF16, tag=f"x{cc}_{b_}",
                                     name=f"x{cc}_{b_}")
                    xs2[cc][b_] = t_
            for cc in range(CC):
                nc.sync.dma_start(xs2[cc][0][:], xT[cc][:, 0:512])
            ensure_wqk("q", 0)
            ensure_wqk("k", 0)
            nc.sync.dma_start(csk[:], cs2[:])
            nc.sync.dma_start(snk[:], sn2[:])
            # dummy exp right after csk arrives: walrus puts the ~2.7 us act
            # table load here, off the critical path
            nc.scalar.activation(dexp[:], csk[0:1, 0:16], EXP, scale=1.0)
            for cc in range(CC):
                nc.sync.dma_start(xs2[cc][1][:], xT[cc][:, ts(1, 512)])
            load_vw(0, 1)
            for b_ in range(2, NB):
                for cc in range(CC):
                    nc.sync.dma_start(xs2[cc][b_][:], xT[cc][:, ts(b_, 512)])
            load_vw(1, 0)
            load_vw(1, 1)

            # block-progressive emission: the bootstrap is DMA-bound (~300
            # GB/s), so spend ~13.6 us of PE work per 2 MB x block instead
            # of burning all V chunks (6.8 us/block) first and stalling.
            v_sb[0] = vg_pool.tile([128, KC, 512], BF16, tag="vg", name="vg0")
            q_sb[0] = q_pool.tile([128, T], BF16, tag="q", name="qh0")
            k_sb[0] = k_pool.tile([128, T], BF16, tag="k", name="kh0")
            for bb in range(NB):
                emit_v_chunk(0, 2 * bb)
                emit_v_chunk(0, 2 * bb + 1)
                emit_proj_half("q", 0, bb // 2, bb % 2)
                emit_proj_half("k", 0, bb // 2, bb % 2)

            # ---------------- main loop: SDPA per head ----------------
            for h in range(NH):
                g, j = h // 4, h % 4
                prod = producer_units(h + 1) if h + 1 < NH else iter(())
                for qt in range(4):
                    qsl = q_sb[h][:, ts(qt, 512)]
                    esum = es_pool.tile([128, 512], BF16, tag="es")
                    o_ps = ps_o.tile([128, 512], F32, tag="pso")
                    eTs = [None] * KC

                    def pv(kc):
                        nc.tensor.matmul(
                            o_ps[:], v_sb[g][:, kc, ts(j, 128)], eTs[kc][:],
                            start=(kc == 0), stop=(kc == KC - 1))

                    for kc in range(KC):
                        s_ps = ps_s.tile([128, 512], F32, tag="pss")
                        nc.tensor.matmul(s_ps[:], k_sb[h][:, ts(kc, 128)], qsl,
                                         start=True, stop=True)
                        eT = eT_pool.tile([128, 512], BF16, tag="eT")
                        nc.scalar.activation(eT[:], s_ps[:], EXP, scale=SCALE)
                        eTs[kc] = eT
                        # bf16 chunk-sum (magnitude ~30; the 2048-wide key
                        # reduction happens exactly in f32 on gpsimd below)
                        with nc.allow_low_precision(reason="bf16 chunk sum"):
                            if kc == 0:
                                nc.vector.tensor_copy(esum[:], eT[:])
                            else:
                                nc.vector.tensor_add(esum[:], esum[:], eT[:])
                        if kc >= 2:
                            pv(kc - 2)
                        if h < NH - 1 and kc in (5, 11):
                            unit = next(prod, None)
                            if unit is not None:
                                emit_proj_half(*unit)
                        if h == NH - 1:
                            if qt == 0 and kc == 1:
                                load_wp_slab(0)
                            elif qt == 0 and kc == 5:
                                load_wp_slab(1)
                            elif (qt, kc) in OP_SLOTS:
                                fc_ = OP_SLOTS[(qt, kc)]
                                emit_op_unit(fc_, 0, "v")
                                if fc_ + 2 < 10:
                                    load_wp_slab(fc_ + 2)
                    pv(KC - 2)
                    pv(KC - 1)
                    # softmax denominator: f32 partition all-reduce on gpsimd
                    # (every partition gets the 512 per-token sums), DVE
                    # reciprocal, then one fused (o * recip) normalize from
                    # PSUM into the f-major attention output staging tile.
                    eb = eb_pool.tile([128, 512], F32, tag="eb")
                    nc.gpsimd.partition_all_reduce(
                        eb[:], esum[:], 128, bass_isa.ReduceOp.add)
                    rb = rb_pool.tile([128, 512], BF16, tag="rb")
                    with nc.allow_low_precision(reason="bf16 softmax denom"):
                        nc.vector.reciprocal(rb[:], eb[:])
                        nc.vector.scalar_tensor_tensor(
                            oT[:, h, ts(qt, 512)], o_ps[:], 1.0, rb[:],
                            MULT, MULT)

                # V(g1) block between head 3 and head 4: v_sb[0] reads are
                # all emitted by now, so the single vg buffer can recycle.
                if h == 3:
                    v_sb[1] = vg_pool.tile([128, KC, 512], BF16, tag="vg",
                                           name="vg1")
                    for tp in range(8):
                        emit_v_chunk(1, tp)

            # ------------- rest of the output projection -------------
            # tq0 tail (slabs 10..15 + second-round 0,1 prefetched 2 ahead)
            for fc in range(8, CC):
                if fc + 2 < CC:
                    load_wp_slab(fc + 2)
                else:
                    load_wp_slab(fc + 2 - CC)
                emit_op_unit_alt(fc, 0)
            # tq 1..3 fc-major: one slab load per fc serves three units
            for fc in range(CC):
                for i, tq in enumerate((1, 2, 3)):
                    if i == 0 and fc + 2 < CC:
                        load_wp_slab(fc + 2)
                    emit_op_unit_alt(fc, tq)

    # populate .instr bytes for extended-inst InstISA subclasses
    # (partition_all_reduce) — raw Bass doesn't run this pass and the NEFF
    # compiler rejects empty .instr with "ISA wrong length"
    mybir.codegen_inst_isa_subclasses(nc)
    _split_multi_waits(nc)
    return nc


# ---------------------------------------------------------------------------
# host-side prep / assembly
# ---------------------------------------------------------------------------


def _to_bf16(a):
    return np.ascontiguousarray(a.astype(ml_dtypes.bfloat16))


def prep_inputs(x, w_attn, w_proj):
    x = np.asarray(x, dtype=np.float32)
    w_attn = np.asarray(w_attn, dtype=np.float32)
    w_proj = np.asarray(w_proj, dtype=np.float32)

    perm = np.concatenate([np.arange(0, HD, 2), np.arange(1, HD, 2)])

    inv = 1.0 / (10000.0 ** (np.arange(0, HD, 2, dtype=np.float64) / HD))
    fr = np.outer(np.arange(T, dtype=np.float64), inv)
    cos = np.cos(fr).T
    sin = np.sin(fr).T
    cs2 = _to_bf16(np.concatenate([cos, cos], 0))
    sn2 = _to_bf16(np.concatenate([-sin, sin], 0))

    # per head-half weight slabs (shared across batches)
    half_slabs = []
    for hh in range(2):
        heads = range(hh * NH, (hh + 1) * NH)
        # wq/wk: [NH, 128p(c within cc), CC*128f] with rope perm on f
        wq_sl = np.empty((NH, 128, CC * 128), dtype=np.float32)
        wk_sl = np.empty((NH, 128, CC * 128), dtype=np.float32)
        for jj, h in enumerate(heads):
            wq_h = w_attn[h * HD:(h + 1) * HD][perm, :]        # [128f, 2048c]
            wk_h = w_attn[D + h * HD:D + (h + 1) * HD][perm, :]
            # slab[p, cc, f] = w[f, cc*128+p]
            wq_sl[jj] = wq_h.T.reshape(CC, 128, 128).transpose(1, 0, 2).reshape(128, -1)
            wk_sl[jj] = wk_h.T.reshape(CC, 128, 128).transpose(1, 0, 2).reshape(128, -1)
        # wv: [2 groups, 2 halves, 128p, 8cc*(4h*128)]
        wv_sl = np.empty((2, 2, 128, 8 * 512), dtype=np.float32)
        for g in range(2):
            hv = w_attn[2 * D + (hh * NH + g * 4) * HD:
                        2 * D + (hh * NH + (g + 1) * 4) * HD]  # [512f, 2048c]
            # [cc, p, f] -> [half, 128p, 8cc, 512f]
            arr = hv.T.reshape(CC, 128, 512)
            for half in range(2):
                wv_sl[g, half] = (arr[half * 8:(half + 1) * 8]
                                  .transpose(1, 0, 2).reshape(128, -1))
        # wp: [128p(hd within hc), fc, hc, 128f]
        #   value = w_proj[fc*128+f, hh*1024 + hc*128 + p]
        wp_cols = w_proj[:, hh * NH * HD:(hh + 1) * NH * HD]  # [2048f, 1024hd]
        wp_sl = (wp_cols.T.reshape(NH, 128, CC, 128)
                 .transpose(1, 2, 0, 3))                       # [128p, fc, hc, f]
        half_slabs.append((_to_bf16(wq_sl), _to_bf16(wk_sl), _to_bf16(wv_sl),
                           _to_bf16(np.ascontiguousarray(wp_sl))))

    xTs = []
    for b in range(B):
        xT = x[b].T.reshape(CC, 128, T)
        xTs.append(_to_bf16(xT))

    in_maps = []
    for i in range(N_CORES):
        b, hh = i // 2, i % 2
        wq_sl, wk_sl, wv_sl, wp_sl = half_slabs[hh]
        in_maps.append({
            "xT": xTs[b],
            "wqs": wq_sl, "wks": wk_sl, "wvs": wv_sl, "wps": wp_sl,
            "cs2": cs2, "sn2": sn2,
        })
    return in_maps


def assemble(results):
    out = np.empty((B, T, D), dtype=np.float32)
    for b in range(B):
        p0 = results[2 * b]["out"].astype(np.float32)
        p1 = results[2 * b + 1]["out"].astype(np.float32)
        out[b] = (p0 + p1).T
    return out


_nc_cache = None


def _get_nc():
    global _nc_cache
    if _nc_cache is None:
        _nc_cache = build_nc()
    return _nc_cache


def kernel(x, w_attn, w_proj):
    from concourse.bass_utils import run_bass_kernel_spmd
    nc = _get_nc()
    in_maps = prep_inputs(x, w_attn, w_proj)
    res = run_bass_kernel_spmd(nc, in_maps, list(range(N_CORES)))
    return assemble(res.results)


def run_profiled(x, w_attn, w_proj, trace_cores=None):
    """Like kernel() but with NTFF profiling; returns BassKernelResults."""
    from concourse.bass_utils import run_bass_kernel_spmd
    import sys as _sys, types as _types
    try:
        import antenv
        if "antenv.axon_hooks" not in _sys.modules:
            mod = _types.ModuleType("antenv.axon_hooks")
            _h = [None]
            mod.set_axon_ntff_profile_hook = lambda h: _h.__setitem__(0, h)
            mod.get_axon_ntff_profile_hook = lambda: _h[0]
            _sys.modules["antenv.axon_hooks"] = mod
            antenv.axon_hooks = mod
            from trn_agent_boot.trn_boot import _ntff_profile_via_ctypes
            mod.set_axon_ntff_profile_hook(
                _ntff_profile_via_ctypes('/opt/axon/libaxon_pjrt.so'))
    except Exception as e:  # profiling is best-effort
        print("profile hook setup failed:", e)
    nc = _get_nc()
    in_maps = prep_inputs(x, w_attn, w_proj)
    return run_bass_kernel_spmd(
        nc, in_maps, list(range(N_CORES)), trace=True,
        trace_cores=trace_cores if trace_cores is not None else [0])


# revision 31
# speedup vs baseline: 1.1574x; 1.1153x over previous
"""8-core Trainium2 Bass kernel v3 for nn_Attention_89489938579587.

reference: qkv = x @ w_attn.T; split q,k,v per 16 heads (HD=128); RoPE
(interleaved pairs); non-causal SDPA; y @ w_proj.T.  B=4, T=2048, D=2048.

Sharding: core i -> (batch b=i//2, head-half hh=i%2).  Each core computes
QKV for its 8 heads over ALL 2048 tokens, RoPE, SDPA, and a PARTIAL output
projection (contraction over its 1024 head-dims) in f-major layout
[2048 f, 2048 t].  Host adds the two partials per batch and transposes.
Ideal 1/8 compute share (25.8 GMAC/core = 3072 N=512 matmuls); no
collectives.

v3 changes over v2 (778 us):
 - softmax denominator reduce + broadcast moved off the PE: gpsimd
   partition_all_reduce + DVE reciprocal + fused scalar_tensor_tensor
   normalize (removes 64 aux/bc matmuls = ~22 us PE busy + disruption).
 - output projection restarted tq-major and started EARLY: 8 units
   interleaved into head-7's SDPA (PE filler while the exp chain drains;
   kills the 2.5-2.9 us/qt stalls + HAM 4/8 oscillation seen in the v2
   trace tail).  w_proj fully SBUF-resident after x is freed at h3.
 - bootstrap: x DMA'd in 512-token blocks with weight DMAs front-loaded
   so the first matmul starts ~7 us (was 16.8); junk matmuls on a memset
   tile pre-warm the HAM clock gate; dummy exp preloads the act table.
 - producer psum->sbuf copies moved from scalar to vector so the scalar
   engine only runs exp (it was the binding engine in the tail).
"""

import numpy as np
from contextlib import ExitStack

import concourse.bass as bass
import concourse.tile as tile
from concourse import mybir
from concourse import bass_isa
from concourse import library_config
from concourse.bass import ts

import bass_rust
import ml_dtypes

# ---------------------------------------------------------------------------
# Toolchain workarounds (same as baseline): walrus rejects >1 sem wait per
# instruction; split extras onto same-engine nops; patch tile drain.
# ---------------------------------------------------------------------------


def _split_multi_waits(nc, max_waits=1):
    n = 0
    for fn in nc.m.functions:
        for blk in fn.blocks:
            insts = blk.instructions
            i = 0
            while i < len(insts):
                inst = insts[i]
                si = inst.sync_info
                waits = list(si.on_wait) if (si is not None and si.on_wait) else []
                if len(waits) > max_waits:
                    si.on_wait = waits[:max_waits]
                    extra = waits[max_waits:]
                    for j in range(0, len(extra), max_waits):
                        nop = mybir.InstNoOp(
                            name=nc.get_next_instruction_name(), ins=[], outs=[])
                        nop.engine = inst.engine
                        nop.sync_info = bass_rust.SyncInfo(
                            on_wait=extra[j:j + max_waits], on_update=[])
                        nc.register_instruction(nop, overwrite=True)
                        insts.insert(i, nop)
                        i += 1
                        n += 1
                i += 1
    return n


def _patched_drain_and_barrier(self, tick_clock, wait_clock):
    from concourse.vector_clock import ScopedClock
    nc = self.nc
    probe = nc.sync.nop()
    wait_clock.add_sem_waits(probe.ins, ScopedClock({None: tick_clock.global_clock}))
    si = probe.ins.sync_info
    waits = list(si.on_wait or []) if si is not None else []
    if len(waits) > 1:
        si.on_wait = [waits[0]]
        for w in waits[1:]:
            nop = nc.sync.nop()
            nsi = nop.ins.sync_info
            if nsi is None:
                nop.ins.sync_info = bass_rust.SyncInfo(on_wait=[w], on_update=[])
            else:
                nsi.on_wait = [w]
    nc.sync.drain()
    nc.all_engine_barrier()
    assert self.sems is not None
    popped = nc._tile_sem_poison_stack.pop()
    assert popped is self._sem_poison
    nc.clear_and_free_semaphores(list(self.sems.allocated().values()))
    nc.all_engine_barrier()


_patched = False


def _apply_patches():
    global _patched
    if not _patched:
        tile.TileContext._drain_and_barrier = _patched_drain_and_barrier
        _patched = True


# ---------------------------------------------------------------------------
# Problem constants
# ---------------------------------------------------------------------------
BF16 = mybir.dt.bfloat16
F32 = mybir.dt.float32
EXP = mybir.ActivationFunctionType.Exp
MULT = mybir.AluOpType.mult

B, T, D, H, HD = 4, 2048, 2048, 16, 128
CC = D // 128            # 16 contraction chunks
NH = 8                   # heads per core
KC = T // 128            # 16 key chunks
NB = 4                   # 512-token x blocks
SCALE = 1.0 / float(np.sqrt(HD))
N_CORES = 8

# out-proj units interleaved into head-7's SDPA: (qt, kc) -> fc of tq0
OP_SLOTS = {(2, 1): 0, (2, 5): 1, (2, 9): 2, (2, 13): 3,
            (3, 1): 4, (3, 5): 5, (3, 9): 6, (3, 13): 7}


def build_nc(n_cores=N_CORES):
    _apply_patches()
    nc = bass.Bass("TRN2", target_bir_lowering=False, debug=False,
                   num_devices=n_cores)
    xT = nc.dram_tensor("xT", [CC, 128, T], BF16, kind="ExternalInput").ap()
    wqs = nc.dram_tensor("wqs", [NH, 128, CC * 128], BF16, kind="ExternalInput").ap()
    wks = nc.dram_tensor("wks", [NH, 128, CC * 128], BF16, kind="ExternalInput").ap()
    # V weights: [group, cc-half, 128p, 8cc * (4h*128f)]
    wvs = nc.dram_tensor("wvs", [2, 2, 128, 8 * 512], BF16, kind="ExternalInput").ap()
    # out-proj: [128p(hd), fc, hc, 128f]
    wps = nc.dram_tensor("wps", [128, CC, NH, 128], BF16, kind="ExternalInput").ap()
    cs2 = nc.dram_tensor("cs2", [128, T], BF16, kind="ExternalInput").ap()
    sn2 = nc.dram_tensor("sn2", [128, T], BF16, kind="ExternalInput").ap()
    # f-major partial output [2048 f, 2048 t]
    out = nc.dram_tensor("out", [D, T], BF16, kind="ExternalOutput").ap()

    with tile.TileContext(nc) as tc, ExitStack() as octx:
        # gpsimd ucode library with partition_all_reduce (loads at t=0,
        # first use is ~60 us in)
        nc.gpsimd.load_library(library_config.attn)

        cs_pool = octx.enter_context(tc.tile_pool(name="cs", bufs=1))
        csk = cs_pool.tile([128, T], BF16, tag="csk")
        snk = cs_pool.tile([128, T], BF16, tag="snk")
        jw = cs_pool.tile([128, 512], BF16, tag="jw")
        dexp = cs_pool.tile([1, 16], BF16, tag="dexp")

        oT_pool = octx.enter_context(tc.tile_pool(name="oT", bufs=1))
        oT = oT_pool.tile([128, NH, T], BF16, tag="oT")

        with ExitStack() as p1:
            wqk_pool = p1.enter_context(tc.tile_pool(name="wqk", bufs=4))
            wv_pool = p1.enter_context(tc.tile_pool(name="wv", bufs=2))
            vg_pool = p1.enter_context(tc.tile_pool(name="vg", bufs=1))
            q_pool = p1.enter_context(tc.tile_pool(name="q", bufs=2))
            k_pool = p1.enter_context(tc.tile_pool(name="k", bufs=2))
            rp_pool = p1.enter_context(tc.tile_pool(name="rp", bufs=2))
            eT_pool = p1.enter_context(tc.tile_pool(name="eT", bufs=8))
            es_pool = p1.enter_context(tc.tile_pool(name="es", bufs=2))
            eb_pool = p1.enter_context(tc.tile_pool(name="eb", bufs=2))
            rb_pool = p1.enter_context(tc.tile_pool(name="rb", bufs=2))
            ob_pool = p1.enter_context(tc.tile_pool(name="ob", bufs=6))
            wpf_pool = p1.enter_context(tc.tile_pool(name="wpf", bufs=4))
            # 8 PSUM banks: qkv/out-proj units share one pool (producers are
            # done before the out-proj starts); ps_o double-buffered so the
            # slow gpsimd norm chain reads bank A while qt+1 accumulates in
            # bank B (chain gets ~28 us of slack, PE never waits on it)
            ps_qkv = p1.enter_context(tc.tile_pool(name="psqkv", bufs=2, space="PSUM"))
            ps_s = p1.enter_context(tc.tile_pool(name="pss", bufs=4, space="PSUM"))
            ps_o = p1.enter_context(tc.tile_pool(name="pso", bufs=2, space="PSUM"))

            x_pool = p1.enter_context(tc.tile_pool(name="x", bufs=1))

            # x resident: 16 [128, 2048] tiles.  One dma_start per cc tile —
            # the sync engine issues DMAs serially at ~650 ns each, so fewer,
            # bigger issues beat fine-grained blocks (v4 measured 57 GB/s
            # effective with 64 block DMAs: issue-bound, queues 7% busy).
            xs = [None] * CC

            # per-core state holders
            q_sb = [None] * NH
            k_sb = [None] * NH
            v_sb = [None] * 2   # per group
            v_w = [[None, None], [None, None]]
            w_hold = {}
            sf_hold = {}
            wp_slabs = {}

            def load_wp_slab(fc):
                w_ = wpf_pool.tile([128, NH, 128], BF16, tag="wpf",
                                   name=f"wpf{fc}")
                nc.sync.dma_start(w_[:], wps[:, fc])
                wp_slabs[fc] = w_

            def load_vw(g, half):
                w_ = wv_pool.tile([128, 8 * 512], BF16, tag="wv")
                nc.sync.dma_start(w_[:], wvs[g, half])
                v_w[g][half] = w_

            def ensure_wqk(kind, h):
                if (kind, h) not in w_hold:
                    wsl = wqk_pool.tile([128, CC, 128], BF16, tag="wqk")
                    nc.sync.dma_start(wsl[:], (wqs if kind == "q" else wks)[h])
                    w_hold[(kind, h)] = wsl
                return w_hold[(kind, h)]

            def emit_proj_half(kind, h, tpair, half):
                """One 16-matmul unit: 512 tokens of a Q/K projection.  After
                the second half of a token-pair, the RoPE chain is emitted."""
                wsl = ensure_wqk(kind, h)
                toff = tpair * 1024
                if half == 0:
                    sf_hold[kind] = rp_pool.tile([128, 1024], BF16, tag="sf",
                                                 name=f"sf_{kind}{h}_{tpair}")
                sf = sf_hold[kind]
                ps = ps_qkv.tile([128, 512], F32, tag="psqkv")
                for cc in range(CC):
                    nc.tensor.matmul(
                        ps[:], wsl[:, cc, :],
                        xs[cc][:, toff + half * 512: toff + (half + 1) * 512],
                        start=(cc == 0), stop=(cc == CC - 1))
                # scalar copy: keeps the psum->sbuf copy out of the DVE FIFO
                # where it sat behind RoPE ops and gated the unit 2 later
                nc.scalar.copy(sf[:, ts(half, 512)], ps[:])
                if half == 0:
                    return
                sw = rp_pool.tile([128, 1024], BF16, tag="sw")
                nc.sync.dma_start(sw[0:64, :], sf[64:128, :])
                nc.sync.dma_start(sw[64:128, :], sf[0:64, :])
                nc.vector.tensor_mul(sf[:], sf[:], csk[:, toff:toff + 1024])
                nc.vector.tensor_mul(sw[:], sw[:], snk[:, toff:toff + 1024])
                dst = q_sb[h] if kind == "q" else k_sb[h]
                nc.vector.tensor_add(dst[:, toff:toff + 1024], sf[:], sw[:])

            def producer_units(hn):
                """Generator of head-hn QKV producer units (8 per head)."""
                q_sb[hn] = q_pool.tile([128, T], BF16, tag="q", name=f"qh{hn}")
                k_sb[hn] = k_pool.tile([128, T], BF16, tag="k", name=f"kh{hn}")
                for kind in ("q", "k"):
                    for tpair in range(2):
                        for half in range(2):
                            yield (kind, hn, tpair, half)

            def emit_v_chunk(g, tch_pair):
                """V for head-group g, two token chunks (2*128 tokens)."""
                for u in range(2):
                    tch = tch_pair * 2 + u
                    ps = ps_qkv.tile([128, 512], F32, tag="psqkv")
                    for cc in range(CC):
                        wv_ap = v_w[g][cc // 8][:, (cc % 8) * 512:(cc % 8 + 1) * 512]
                        nc.tensor.matmul(
                            ps[:], xs[cc][:, ts(tch, 128)], wv_ap,
                            start=(cc == 0), stop=(cc == CC - 1))
                    if tch % 2 == 0:
                        nc.scalar.copy(v_sb[g][:, tch, :], ps[:])
                    else:
                        nc.vector.tensor_copy(v_sb[g][:, tch, :], ps[:])

            op_state = {"alt": 0}

            def emit_op_unit(fc, tq, eng):
                """Partial out-proj: one [128f x 512t] tile, contraction over
                this core's 8 heads."""
                wp = wp_slabs[fc]
                ps = ps_qkv.tile([128, 512], F32, tag="psqkv")
                for hc in range(NH):
                    nc.tensor.matmul(ps[:], wp[:, hc, :],
                                     oT[:, hc, ts(tq, 512)],
                                     start=(hc == 0), stop=(hc == NH - 1))
                oe = ob_pool.tile([128, 512], BF16, tag="ob")
                if eng == "v":
                    nc.vector.tensor_copy(oe[:], ps[:])
                else:
                    nc.scalar.copy(oe[:], ps[:])
                nc.sync.dma_start(out[ts(fc, 128), ts(tq, 512)], oe[:])

            def emit_op_unit_alt(fc, tq):
                op_state["alt"] ^= 1
                emit_op_unit(fc, tq, "s" if op_state["alt"] else "v")

            # ---------------- bootstrap ----------------
            # HAM warm-up: junk matmuls on a memset tile while x DMA lands.
            nc.vector.memset(jw[:], 0.0)
            jp = ps_qkv.tile([128, 512], F32, tag="psqkv", name="junk")
            for i in range(14):
                nc.tensor.matmul(jp[:], jw[:, 0:128], jw[:],
                                 start=True, stop=True)

            # DMA order = sync-queue order.  The sync engine issues DMAs
            # serially at ~1.3 us each, so small high-priority loads go
            # first; wv(1,*) reuse slots whose WAR wait would BLOCK the
            # queue, so they are emitted after the bootstrap (see below).
            load_vw(0, 0)
            ensure_wqk("q", 0)
            ensure_wqk("k", 0)
            nc.sync.dma_start(csk[:], cs2[:])
            nc.sync.dma_start(snk[:], sn2[:])
            # dummy exp right after csk arrives: walrus puts the ~2.7 us act
            # table load here, off the critical path
            nc.scalar.activation(dexp[:], csk[0:1, 0:16], EXP, scale=1.0)
            for cc in range(CC):
                t_ = x_pool.tile([128, T], BF16, tag=f"x{cc}", name=f"x{cc}")
                xs[cc] = t_
            for cc in range(8):
                nc.sync.dma_start(xs[cc][:], xT[cc])
            load_vw(0, 1)
            for cc in range(8, CC):
                nc.sync.dma_start(xs[cc][:], xT[cc])

            # block-progressive emission: the bootstrap is DMA-bound (~300
            # GB/s), so spend ~13.6 us of PE work per 2 MB x block instead
            # of burning all V chunks (6.8 us/block) first and stalling.
            v_sb[0] = vg_pool.tile([128, KC, 512], BF16, tag="vg", name="vg0")
            q_sb[0] = q_pool.tile([128, T], BF16, tag="q", name="qh0")
            k_sb[0] = k_pool.tile([128, T], BF16, tag="k", name="kh0")
            for bb in range(NB):
                emit_v_chunk(0, 2 * bb)
                emit_v_chunk(0, 2 * bb + 1)
                emit_proj_half("q", 0, bb // 2, bb % 2)
                emit_proj_half("k", 0, bb // 2, bb % 2)
            # h1 slabs (fresh wqk slots, no WAR wait) then the g1 V weights,
            # whose slot-reuse WAR wait blocks the sync queue until the g0
            # V matmuls have all read their slabs — nothing urgent behind.
            ensure_wqk("q", 1)
            ensure_wqk("k", 1)
            load_vw(1, 0)
            load_vw(1, 1)

            # Deferred softmax normalization: the gpsimd all-reduce takes
            # ~3.6 us, and the DVE queue is strict FIFO — an immediately-
            # emitted reciprocal would block the next qt's esum adds behind
            # the all-reduce (measured 4.9 us PE gap per qt).  Defer the
            # recip+normalize to kc==10 of the NEXT qt, when the all-reduce
            # result is long ready.
            pending_norms = []

            def emit_norm(h_, qt_, o_ps_t, eb_t):
                rb = rb_pool.tile([128, 512], BF16, tag="rb")
                with nc.allow_low_precision(reason="bf16 softmax denom"):
                    nc.vector.reciprocal(rb[:], eb_t[:])
                    nc.vector.scalar_tensor_tensor(
                        oT[:, h_, ts(qt_, 512)], o_ps_t[:], 1.0, rb[:],
                        MULT, MULT)

            def pop_norm(lag=0):
                if len(pending_norms) > lag:
                    emit_norm(*pending_norms.pop(0))

            # ---------------- main loop: SDPA per head ----------------
            for h in range(NH):
                g, j = h // 4, h % 4
                prod = producer_units(h + 1) if h + 1 < NH else iter(())
                if h + 1 < NH:
                    # prefetch next head's Q/K slabs so the kc==1 producer
                    # unit never waits on the ~1.3 us/DMA sync issue queue
                    ensure_wqk("q", h + 1)
                    ensure_wqk("k", h + 1)
                for qt in range(4):
                    qsl = q_sb[h][:, ts(qt, 512)]
                    esum = es_pool.tile([128, 512], BF16, tag="es")
                    o_ps = ps_o.tile([128, 512], F32, tag="pso")
                    eTs = [None] * KC

                    def pv(kc):
                        nc.tensor.matmul(
                            o_ps[:], v_sb[g][:, kc, ts(j, 128)], eTs[kc][:],
                            start=(kc == 0), stop=(kc == KC - 1))

                    for kc in range(KC):
                        s_ps = ps_s.tile([128, 512], F32, tag="pss")
                        nc.tensor.matmul(s_ps[:], k_sb[h][:, ts(kc, 128)], qsl,
                                         start=True, stop=True)
                        eT = eT_pool.tile([128, 512], BF16, tag="eT")
                        nc.scalar.activation(eT[:], s_ps[:], EXP, scale=SCALE)
                        eTs[kc] = eT
                        # bf16 chunk-sum (magnitude ~30; the 2048-wide key
                        # reduction happens exactly in f32 on gpsimd below)
                        with nc.allow_low_precision(reason="bf16 chunk sum"):
                            if kc == 0:
                                nc.vector.tensor_copy(esum[:], eT[:])
                            else:
                                nc.vector.tensor_add(esum[:], esum[:], eT[:])
                        if kc >= 2:
                            pv(kc - 2)
                        if kc == 10:
                            pop_norm()
                        if h < NH - 1 and kc in (1, 7):
                            # slots early in the qt so the last unit's RoPE
                            # chain lands before the next head's first scores
                            unit = next(prod, None)
                            if unit is not None:
                                emit_proj_half(*unit)
                        if h == NH - 1:
                            if qt == 0 and kc == 1:
                                load_wp_slab(0)
                            elif qt == 0 and kc == 5:
                                load_wp_slab(1)
                            elif (qt, kc) in OP_SLOTS:
                                fc_ = OP_SLOTS[(qt, kc)]
                                emit_op_unit_alt(fc_, 0)
                                if fc_ + 2 < 10:
                                    load_wp_slab(fc_ + 2)
                            elif qt in (0, 1) and kc in (6, 12):
                                # no fillable work exists yet in h7's first
                                # two qts (scalar-bound); burn one junk MM
                                # so PE idle gaps stay under the ~3.4 us
                                # HAM re-throttle window
                                jp_ = ps_qkv.tile([128, 512], F32,
                                                  tag="psqkv",
                                                  name=f"junk7_{qt}_{kc}")
                                nc.tensor.matmul(jp_[:], jw[:, 0:128], jw[:],
                                                 start=True, stop=True)
                    pv(KC - 2)
                    pv(KC - 1)
                    # softmax denominator: f32 partition all-reduce on gpsimd
                    # (every partition gets the 512 per-token sums); the
                    # recip + (o * recip) normalize is deferred (see above).
                    eb = eb_pool.tile([128, 512], F32, tag="eb")
                    nc.gpsimd.partition_all_reduce(
                        eb[:], esum[:], 128, bass_isa.ReduceOp.add)
                    pending_norms.append((h, qt, o_ps, eb))

                # V(g1) block between head 3 and head 4: v_sb[0] reads are
                # all emitted by now, so the single vg buffer can recycle.
                if h == 3:
                    v_sb[1] = vg_pool.tile([128, KC, 512], BF16, tag="vg",
                                           name="vg1")
                    for tp in range(8):
                        emit_v_chunk(1, tp)

            # ------------- rest of the output projection -------------
            # tq0 tail (slabs 10..15 + second-round 0,1 prefetched 2 ahead);
            # the last pending norm (7,qt3) flushes once its all-reduce has
            # had ~5 us to finish
            for fc in range(8, CC):
                if fc + 2 < CC:
                    load_wp_slab(fc + 2)
                else:
                    load_wp_slab(fc + 2 - CC)
                emit_op_unit_alt(fc, 0)
                if fc == 10:
                    while pending_norms:
                        emit_norm(*pending_norms.pop(0))
            # tq 1..3 fc-major: one slab load per fc serves three units
            for fc in range(CC):
                for i, tq in enumerate((1, 2, 3)):
                    if i == 0 and fc + 2 < CC:
                        load_wp_slab(fc + 2)
                    emit_op_unit_alt(fc, tq)

    # populate .instr bytes for extended-inst InstISA subclasses
    # (partition_all_reduce) — raw Bass doesn't run this pass and the NEFF
    # compiler rejects empty .instr with "ISA wrong length"
    mybir.codegen_inst_isa_subclasses(nc)
    _split_multi_waits(nc)
    return nc


# ---------------------------------------------------------------------------
# host-side prep / assembly
# ---------------------------------------------------------------------------


def _to_bf16(a):
    return np.ascontiguousarray(a.astype(ml_dtypes.bfloat16))


def prep_inputs(x, w_attn, w_proj):
    x = np.asarray(x, dtype=np.float32)
    w_attn = np.asarray(w_attn, dtype=np.float32)
    w_proj = np.asarray(w_proj, dtype=np.float32)

    perm = np.concatenate([np.arange(0, HD, 2), np.arange(1, HD, 2)])

    inv = 1.0 / (10000.0 ** (np.arange(0, HD, 2, dtype=np.float64) / HD))
    fr = np.outer(np.arange(T, dtype=np.float64), inv)
    cos = np.cos(fr).T
    sin = np.sin(fr).T
    cs2 = _to_bf16(np.concatenate([cos, cos], 0))
    sn2 = _to_bf16(np.concatenate([-sin, sin], 0))

    # per head-half weight slabs (shared across batches)
    half_slabs = []
    for hh in range(2):
        heads = range(hh * NH, (hh + 1) * NH)
        # wq/wk: [NH, 128p(c within cc), CC*128f] with rope perm on f
        wq_sl = np.empty((NH, 128, CC * 128), dtype=np.float32)
        wk_sl = np.empty((NH, 128, CC * 128), dtype=np.float32)
        for jj, h in enumerate(heads):
            wq_h = w_attn[h * HD:(h + 1) * HD][perm, :]        # [128f, 2048c]
            wk_h = w_attn[D + h * HD:D + (h + 1) * HD][perm, :]
            # slab[p, cc, f] = w[f, cc*128+p]
            wq_sl[jj] = wq_h.T.reshape(CC, 128, 128).transpose(1, 0, 2).reshape(128, -1)
            wk_sl[jj] = wk_h.T.reshape(CC, 128, 128).transpose(1, 0, 2).reshape(128, -1)
        # wv: [2 groups, 2 halves, 128p, 8cc*(4h*128)]
        wv_sl = np.empty((2, 2, 128, 8 * 512), dtype=np.float32)
        for g in range(2):
            hv = w_attn[2 * D + (hh * NH + g * 4) * HD:
                        2 * D + (hh * NH + (g + 1) * 4) * HD]  # [512f, 2048c]
            # [cc, p, f] -> [half, 128p, 8cc, 512f]
            arr = hv.T.reshape(CC, 128, 512)
            for half in range(2):
                wv_sl[g, half] = (arr[half * 8:(half + 1) * 8]
                                  .transpose(1, 0, 2).reshape(128, -1))
        # wp: [128p(hd within hc), fc, hc, 128f]
        #   value = w_proj[fc*128+f, hh*1024 + hc*128 + p]
        wp_cols = w_proj[:, hh * NH * HD:(hh + 1) * NH * HD]  # [2048f, 1024hd]
        wp_sl = (wp_cols.T.reshape(NH, 128, CC, 128)
                 .transpose(1, 2, 0, 3))                       # [128p, fc, hc, f]
        half_slabs.append((_to_bf16(wq_sl), _to_bf16(wk_sl), _to_bf16(wv_sl),
                           _to_bf16(np.ascontiguousarray(wp_sl))))

    xTs = []
    for b in range(B):
        xT = x[b].T.reshape(CC, 128, T)
        xTs.append(_to_bf16(xT))

    in_maps = []
    for i in range(N_CORES):
        b, hh = i // 2, i % 2
        wq_sl, wk_sl, wv_sl, wp_sl = half_slabs[hh]
        in_maps.append({
            "xT": xTs[b],
            "wqs": wq_sl, "wks": wk_sl, "wvs": wv_sl, "wps": wp_sl,
            "cs2": cs2, "sn2": sn2,
        })
    return in_maps


def assemble(results):
    out = np.empty((B, T, D), dtype=np.float32)
    for b in range(B):
        p0 = results[2 * b]["out"].astype(np.float32)
        p1 = results[2 * b + 1]["out"].astype(np.float32)
        out[b] = (p0 + p1).T
    return out


_nc_cache = None


def _get_nc():
    global _nc_cache
    if _nc_cache is None:
        _nc_cache = build_nc()
    return _nc_cache


def kernel(x, w_attn, w_proj):
    from concourse.bass_utils import run_bass_kernel_spmd
    nc = _get_nc()
    in_maps = prep_inputs(x, w_attn, w_proj)
    res = run_bass_kernel_spmd(nc, in_maps, list(range(N_CORES)))
    return assemble(res.results)


def run_profiled(x, w_attn, w_proj, trace_cores=None):
    """Like kernel() but with NTFF profiling; returns BassKernelResults."""
    from concourse.bass_utils import run_bass_kernel_spmd
    import sys as _sys, types as _types
    try:
        import antenv
        if "antenv.axon_hooks" not in _sys.modules:
            mod = _types.ModuleType("antenv.axon_hooks")
            _h = [None]
            mod.set_axon_ntff_profile_hook = lambda h: _h.__setitem__(0, h)
            mod.get_axon_ntff_profile_hook = lambda: _h[0]
            _sys.modules["antenv.axon_hooks"] = mod
            antenv.axon_hooks = mod
            from trn_agent_boot.trn_boot import _ntff_profile_via_ctypes
            mod.set_axon_ntff_profile_hook(
                _ntff_profile_via_ctypes('/opt/axon/libaxon_pjrt.so'))
    except Exception as e:  # profiling is best-effort
        print("profile hook setup failed:", e)
    nc = _get_nc()
    in_maps = prep_inputs(x, w_attn, w_proj)
    return run_bass_kernel_spmd(
        nc, in_maps, list(range(N_CORES)), trace=True,
        trace_cores=trace_cores if trace_cores is not None else [0])


# revision 36
# speedup vs baseline: 1.2866x; 1.1116x over previous
"""8-core Trainium2 Bass kernel v3 for nn_Attention_89489938579587.

reference: qkv = x @ w_attn.T; split q,k,v per 16 heads (HD=128); RoPE
(interleaved pairs); non-causal SDPA; y @ w_proj.T.  B=4, T=2048, D=2048.

Sharding: core i -> (batch b=i//2, head-half hh=i%2).  Each core computes
QKV for its 8 heads over ALL 2048 tokens, RoPE, SDPA, and a PARTIAL output
projection (contraction over its 1024 head-dims) in f-major layout
[2048 f, 2048 t].  Host adds the two partials per batch and transposes.
Ideal 1/8 compute share (25.8 GMAC/core = 3072 N=512 matmuls); no
collectives.

v3 changes over v2 (778 us):
 - softmax denominator reduce + broadcast moved off the PE: gpsimd
   partition_all_reduce + DVE reciprocal + fused scalar_tensor_tensor
   normalize (removes 64 aux/bc matmuls = ~22 us PE busy + disruption).
 - output projection restarted tq-major and started EARLY: 8 units
   interleaved into head-7's SDPA (PE filler while the exp chain drains;
   kills the 2.5-2.9 us/qt stalls + HAM 4/8 oscillation seen in the v2
   trace tail).  w_proj fully SBUF-resident after x is freed at h3.
 - bootstrap: x DMA'd in 512-token blocks with weight DMAs front-loaded
   so the first matmul starts ~7 us (was 16.8); junk matmuls on a memset
   tile pre-warm the HAM clock gate; dummy exp preloads the act table.
 - producer psum->sbuf copies moved from scalar to vector so the scalar
   engine only runs exp (it was the binding engine in the tail).
"""

import numpy as np
from contextlib import ExitStack

import concourse.bass as bass
import concourse.tile as tile
from concourse import mybir
from concourse import bass_isa
from concourse import library_config
from concourse.bass import ts

import bass_rust
import ml_dtypes

# ---------------------------------------------------------------------------
# Toolchain workarounds (same as baseline): walrus rejects >1 sem wait per
# instruction; split extras onto same-engine nops; patch tile drain.
# ---------------------------------------------------------------------------


def _split_multi_waits(nc, max_waits=1):
    n = 0
    for fn in nc.m.functions:
        for blk in fn.blocks:
            insts = blk.instructions
            i = 0
            while i < len(insts):
                inst = insts[i]
                si = inst.sync_info
                waits = list(si.on_wait) if (si is not None and si.on_wait) else []
                if len(waits) > max_waits:
                    si.on_wait = waits[:max_waits]
                    extra = waits[max_waits:]
                    for j in range(0, len(extra), max_waits):
                        nop = mybir.InstNoOp(
                            name=nc.get_next_instruction_name(), ins=[], outs=[])
                        nop.engine = inst.engine
                        nop.sync_info = bass_rust.SyncInfo(
                            on_wait=extra[j:j + max_waits], on_update=[])
                        nc.register_instruction(nop, overwrite=True)
                        insts.insert(i, nop)
                        i += 1
                        n += 1
                i += 1
    return n


def _patched_drain_and_barrier(self, tick_clock, wait_clock):
    from concourse.vector_clock import ScopedClock
    nc = self.nc
    probe = nc.sync.nop()
    wait_clock.add_sem_waits(probe.ins, ScopedClock({None: tick_clock.global_clock}))
    si = probe.ins.sync_info
    waits = list(si.on_wait or []) if si is not None else []
    if len(waits) > 1:
        si.on_wait = [waits[0]]
        for w in waits[1:]:
            nop = nc.sync.nop()
            nsi = nop.ins.sync_info
            if nsi is None:
                nop.ins.sync_info = bass_rust.SyncInfo(on_wait=[w], on_update=[])
            else:
                nsi.on_wait = [w]
    nc.sync.drain()
    nc.all_engine_barrier()
    assert self.sems is not None
    popped = nc._tile_sem_poison_stack.pop()
    assert popped is self._sem_poison
    nc.clear_and_free_semaphores(list(self.sems.allocated().values()))
    nc.all_engine_barrier()


_patched = False


def _apply_patches():
    global _patched
    if not _patched:
        tile.TileContext._drain_and_barrier = _patched_drain_and_barrier
        _patched = True


# ---------------------------------------------------------------------------
# Problem constants
# ---------------------------------------------------------------------------
BF16 = mybir.dt.bfloat16
F32 = mybir.dt.float32
EXP = mybir.ActivationFunctionType.Exp
MULT = mybir.AluOpType.mult

B, T, D, H, HD = 4, 2048, 2048, 16, 128
CC = D // 128            # 16 contraction chunks
NH = 8                   # heads per core
KC = T // 128            # 16 key chunks
NB = 4                   # 512-token x blocks
SCALE = 1.0 / float(np.sqrt(HD))
N_CORES = 8

# out-proj units interleaved into head-7's SDPA: (qt, kc) -> fc of tq0
OP_SLOTS = {(2, 1): 0, (2, 5): 1, (2, 9): 2, (2, 13): 3,
            (3, 1): 4, (3, 5): 5, (3, 9): 6, (3, 13): 7}


def build_nc(n_cores=N_CORES):
    _apply_patches()
    nc = bass.Bass("TRN2", target_bir_lowering=False, debug=False,
                   num_devices=n_cores)
    xT = nc.dram_tensor("xT", [CC, 128, T], BF16, kind="ExternalInput").ap()
    wqs = nc.dram_tensor("wqs", [NH, 128, CC * 128], BF16, kind="ExternalInput").ap()
    wks = nc.dram_tensor("wks", [NH, 128, CC * 128], BF16, kind="ExternalInput").ap()
    # V weights: [group, cc-half, 128p, 8cc * (4h*128f)]
    wvs = nc.dram_tensor("wvs", [2, 2, 128, 8 * 512], BF16, kind="ExternalInput").ap()
    # out-proj: [128p(hd), fc, hc, 128f]
    wps = nc.dram_tensor("wps", [128, CC, NH, 128], BF16, kind="ExternalInput").ap()
    cs2 = nc.dram_tensor("cs2", [128, T], BF16, kind="ExternalInput").ap()
    sn2 = nc.dram_tensor("sn2", [128, T], BF16, kind="ExternalInput").ap()
    # f-major partial output [2048 f, 2048 t]
    out = nc.dram_tensor("out", [D, T], BF16, kind="ExternalOutput").ap()

    with tile.TileContext(nc) as tc, ExitStack() as octx:
        # gpsimd ucode library with partition_all_reduce (loads at t=0,
        # first use is ~60 us in)
        nc.gpsimd.load_library(library_config.attn)

        cs_pool = octx.enter_context(tc.tile_pool(name="cs", bufs=1))
        csk = cs_pool.tile([128, T], BF16, tag="csk")
        snk = cs_pool.tile([128, T], BF16, tag="snk")
        jw = cs_pool.tile([128, 512], BF16, tag="jw")
        dexp = cs_pool.tile([1, 16], BF16, tag="dexp")

        oT_pool = octx.enter_context(tc.tile_pool(name="oT", bufs=1))
        oT = oT_pool.tile([128, NH, T], BF16, tag="oT")

        with ExitStack() as p1:
            wqk_pool = p1.enter_context(tc.tile_pool(name="wqk", bufs=4))
            wv_pool = p1.enter_context(tc.tile_pool(name="wv", bufs=2))
            vg_pool = p1.enter_context(tc.tile_pool(name="vg", bufs=1))
            q_pool = p1.enter_context(tc.tile_pool(name="q", bufs=2))
            k_pool = p1.enter_context(tc.tile_pool(name="k", bufs=2))
            rp_pool = p1.enter_context(tc.tile_pool(name="rp", bufs=2))
            eT_pool = p1.enter_context(tc.tile_pool(name="eT", bufs=8))
            es_pool = p1.enter_context(tc.tile_pool(name="es", bufs=2))
            eb_pool = p1.enter_context(tc.tile_pool(name="eb", bufs=2))
            rb_pool = p1.enter_context(tc.tile_pool(name="rb", bufs=2))
            ob_pool = p1.enter_context(tc.tile_pool(name="ob", bufs=4))
            wpf_pool = p1.enter_context(tc.tile_pool(name="wpf", bufs=4))
            # 8 PSUM banks: qkv/out-proj units share one pool (producers are
            # done before the out-proj starts); ps_o double-buffered so the
            # slow gpsimd norm chain reads bank A while qt+1 accumulates in
            # bank B (chain gets ~28 us of slack, PE never waits on it)
            ps_qkv = p1.enter_context(tc.tile_pool(name="psqkv", bufs=2, space="PSUM"))
            ps_s = p1.enter_context(tc.tile_pool(name="pss", bufs=4, space="PSUM"))
            ps_o = p1.enter_context(tc.tile_pool(name="pso", bufs=2, space="PSUM"))

            x_pool = p1.enter_context(tc.tile_pool(name="x", bufs=1))

            # x resident: 16 [128, 2048] tiles.  One dma_start per cc tile —
            # the sync engine issues DMAs serially at ~650 ns each, so fewer,
            # bigger issues beat fine-grained blocks (v4 measured 57 GB/s
            # effective with 64 block DMAs: issue-bound, queues 7% busy).
            xs = [None] * CC

            # per-core state holders
            q_sb = [None] * NH
            k_sb = [None] * NH
            v_sb = [None] * 2   # per group
            v_w = [[None, None], [None, None]]
            w_hold = {}
            sf_hold = {}
            wp_slabs = {}

            def load_wp_slab(fc):
                w_ = wpf_pool.tile([128, NH, 128], BF16, tag="wpf",
                                   name=f"wpf{fc}")
                nc.sync.dma_start(w_[:], wps[:, fc])
                wp_slabs[fc] = w_

            def load_vw(g, half):
                w_ = wv_pool.tile([128, 8 * 512], BF16, tag="wv")
                nc.sync.dma_start(w_[:], wvs[g, half])
                v_w[g][half] = w_

            def ensure_wqk(kind, h):
                if (kind, h) not in w_hold:
                    wsl = wqk_pool.tile([128, CC, 128], BF16, tag="wqk")
                    nc.sync.dma_start(wsl[:], (wqs if kind == "q" else wks)[h])
                    w_hold[(kind, h)] = wsl
                return w_hold[(kind, h)]

            def emit_proj_half(kind, h, tpair, half):
                """One 16-matmul unit: 512 tokens of a Q/K projection.  After
                the second half of a token-pair, the RoPE chain is emitted."""
                wsl = ensure_wqk(kind, h)
                toff = tpair * 1024
                if half == 0:
                    sf_hold[kind] = rp_pool.tile([128, 1024], BF16, tag="sf",
                                                 name=f"sf_{kind}{h}_{tpair}")
                sf = sf_hold[kind]
                ps = ps_qkv.tile([128, 512], F32, tag="psqkv")
                for cc in range(CC):
                    nc.tensor.matmul(
                        ps[:], wsl[:, cc, :],
                        xs[cc][:, toff + half * 512: toff + (half + 1) * 512],
                        start=(cc == 0), stop=(cc == CC - 1))
                # scalar copy: keeps the psum->sbuf copy out of the DVE FIFO
                # where it sat behind RoPE ops and gated the unit 2 later
                nc.scalar.copy(sf[:, ts(half, 512)], ps[:])
                if half == 0:
                    return
                sw = rp_pool.tile([128, 1024], BF16, tag="sw")
                nc.sync.dma_start(sw[0:64, :], sf[64:128, :])
                nc.sync.dma_start(sw[64:128, :], sf[0:64, :])
                nc.vector.tensor_mul(sf[:], sf[:], csk[:, toff:toff + 1024])
                nc.vector.tensor_mul(sw[:], sw[:], snk[:, toff:toff + 1024])
                dst = q_sb[h] if kind == "q" else k_sb[h]
                nc.vector.tensor_add(dst[:, toff:toff + 1024], sf[:], sw[:])

            def producer_units(hn):
                """Generator of head-hn QKV producer units (8 per head)."""
                q_sb[hn] = q_pool.tile([128, T], BF16, tag="q", name=f"qh{hn}")
                k_sb[hn] = k_pool.tile([128, T], BF16, tag="k", name=f"kh{hn}")
                for kind in ("q", "k"):
                    for tpair in range(2):
                        for half in range(2):
                            yield (kind, hn, tpair, half)

            def emit_v_chunk(g, tch_pair):
                """V for head-group g, two token chunks (2*128 tokens)."""
                for u in range(2):
                    tch = tch_pair * 2 + u
                    ps = ps_qkv.tile([128, 512], F32, tag="psqkv")
                    for cc in range(CC):
                        wv_ap = v_w[g][cc // 8][:, (cc % 8) * 512:(cc % 8 + 1) * 512]
                        nc.tensor.matmul(
                            ps[:], xs[cc][:, ts(tch, 128)], wv_ap,
                            start=(cc == 0), stop=(cc == CC - 1))
                    if tch % 2 == 0:
                        nc.scalar.copy(v_sb[g][:, tch, :], ps[:])
                    else:
                        nc.vector.tensor_copy(v_sb[g][:, tch, :], ps[:])

            op_state = {"alt": 0}

            def emit_op_unit(fc, tq, eng):
                """Partial out-proj: one [128f x 512t] tile, contraction over
                this core's 8 heads."""
                wp = wp_slabs[fc]
                ps = ps_qkv.tile([128, 512], F32, tag="psqkv")
                for hc in range(NH):
                    nc.tensor.matmul(ps[:], wp[:, hc, :],
                                     oT[:, hc, ts(tq, 512)],
                                     start=(hc == 0), stop=(hc == NH - 1))
                oe = ob_pool.tile([128, 512], BF16, tag="ob")
                if eng == "v":
                    nc.vector.tensor_copy(oe[:], ps[:])
                else:
                    nc.scalar.copy(oe[:], ps[:])
                nc.sync.dma_start(out[ts(fc, 128), ts(tq, 512)], oe[:])

            def emit_op_unit_alt(fc, tq):
                op_state["alt"] ^= 1
                emit_op_unit(fc, tq, "s" if op_state["alt"] else "v")

            # ---------------- bootstrap ----------------
            # HAM warm-up: junk matmuls on a memset tile while x DMA lands.
            nc.vector.memset(jw[:], 0.0)
            jp = ps_qkv.tile([128, 512], F32, tag="psqkv", name="junk")
            for i in range(14):
                nc.tensor.matmul(jp[:], jw[:, 0:128], jw[:],
                                 start=True, stop=True)

            # DMA order = sync-queue order.  The sync engine issues DMAs
            # serially at ~1.3 us each, so small high-priority loads go
            # first; wv(1,*) reuse slots whose WAR wait would BLOCK the
            # queue, so they are emitted after the bootstrap (see below).
            # spread the 22 bootstrap DMA issues over both HWDGE-capable
            # queues — sync (SP) and scalar (Activation) — each issues
            # serially at ~1.3 us/DMA
            for cc in range(CC):
                t_ = x_pool.tile([128, T], BF16, tag=f"x{cc}", name=f"x{cc}")
                xs[cc] = t_
            load_vw(0, 0)
            nc.sync.dma_start(xs[0][:], xT[0])
            nc.sync.dma_start(xs[1][:], xT[1])
            ensure_wqk("q", 0)
            ensure_wqk("k", 0)
            nc.sync.dma_start(csk[:], cs2[:])
            nc.sync.dma_start(snk[:], sn2[:])
            for cc in range(2, 8):
                nc.sync.dma_start(xs[cc][:], xT[cc])
            w_ = wv_pool.tile([128, 8 * 512], BF16, tag="wv", name="wv01")
            nc.scalar.dma_start(w_[:], wvs[0, 1])
            v_w[0][1] = w_
            for cc in range(8, CC):
                nc.scalar.dma_start(xs[cc][:], xT[cc])
            # dummy exp after the scalar-queue issues: walrus puts the
            # ~2.7 us act table load here, off the critical path
            nc.scalar.activation(dexp[:], csk[0:1, 0:16], EXP, scale=1.0)

            # block-progressive emission: the bootstrap is DMA-bound (~300
            # GB/s), so spend ~13.6 us of PE work per 2 MB x block instead
            # of burning all V chunks (6.8 us/block) first and stalling.
            v_sb[0] = vg_pool.tile([128, KC, 512], BF16, tag="vg", name="vg0")
            q_sb[0] = q_pool.tile([128, T], BF16, tag="q", name="qh0")
            k_sb[0] = k_pool.tile([128, T], BF16, tag="k", name="kh0")
            for bb in range(NB):
                emit_v_chunk(0, 2 * bb)
                emit_v_chunk(0, 2 * bb + 1)
                emit_proj_half("q", 0, bb // 2, bb % 2)
                emit_proj_half("k", 0, bb // 2, bb % 2)
            # h1 slabs (fresh wqk slots, no WAR wait) then the g1 V weights,
            # whose slot-reuse WAR wait blocks the sync queue until the g0
            # V matmuls have all read their slabs — nothing urgent behind.
            ensure_wqk("q", 1)
            ensure_wqk("k", 1)
            load_vw(1, 0)
            load_vw(1, 1)

            # Deferred softmax normalization: the gpsimd all-reduce takes
            # ~3.6 us, and the DVE queue is strict FIFO — an immediately-
            # emitted reciprocal would block the next qt's esum adds behind
            # the all-reduce (measured 4.9 us PE gap per qt).  Defer the
            # recip+normalize to kc==10 of the NEXT qt, when the all-reduce
            # result is long ready.
            pending_norms = []

            def emit_norm(h_, qt_, o_ps_t, eb_t):
                # approx recip: ~0.7 us vs 3.35 us for nc.vector.reciprocal,
                # ~51 ULP f32 — far better than the bf16 denom it replaces
                rb = rb_pool.tile([128, 512], F32, tag="rb")
                nc.vector.reciprocal_approx_fast(out=rb[:], in_=eb_t[:])
                with nc.allow_low_precision(reason="bf16 attn out"):
                    nc.vector.scalar_tensor_tensor(
                        oT[:, h_, ts(qt_, 512)], o_ps_t[:], 1.0, rb[:],
                        MULT, MULT)

            def pop_norm(lag=0):
                if len(pending_norms) > lag:
                    emit_norm(*pending_norms.pop(0))

            # ---------------- main loop: SDPA per head ----------------
            for h in range(NH):
                g, j = h // 4, h % 4
                prod = producer_units(h + 1) if h + 1 < NH else iter(())
                if h + 1 < NH:
                    # prefetch next head's Q/K slabs so the kc==1 producer
                    # unit never waits on the ~1.3 us/DMA sync issue queue
                    ensure_wqk("q", h + 1)
                    ensure_wqk("k", h + 1)
                for qt in range(4):
                    qsl = q_sb[h][:, ts(qt, 512)]
                    esum = es_pool.tile([128, 512], BF16, tag="es")
                    o_ps = ps_o.tile([128, 512], F32, tag="pso")
                    eTs = [None] * KC

                    def pv(kc):
                        nc.tensor.matmul(
                            o_ps[:], v_sb[g][:, kc, ts(j, 128)], eTs[kc][:],
                            start=(kc == 0), stop=(kc == KC - 1))

                    for kc in range(KC):
                        s_ps = ps_s.tile([128, 512], F32, tag="pss")
                        nc.tensor.matmul(s_ps[:], k_sb[h][:, ts(kc, 128)], qsl,
                                         start=True, stop=True)
                        eT = eT_pool.tile([128, 512], BF16, tag="eT")
                        nc.scalar.activation(eT[:], s_ps[:], EXP, scale=SCALE)
                        eTs[kc] = eT
                        # bf16 chunk-sum (magnitude ~30; the 2048-wide key
                        # reduction happens exactly in f32 on gpsimd below)
                        with nc.allow_low_precision(reason="bf16 chunk sum"):
                            if kc == 0:
                                nc.vector.tensor_copy(esum[:], eT[:])
                            else:
                                nc.vector.tensor_add(esum[:], esum[:], eT[:])
                        if kc >= 2:
                            pv(kc - 2)
                        if kc == 12:
                            pop_norm()
                        if h < NH - 1 and kc in (1, 7):
                            # slots early in the qt so the last unit's RoPE
                            # chain lands before the next head's first scores
                            unit = next(prod, None)
                            if unit is not None:
                                emit_proj_half(*unit)
                        if h == NH - 1:
                            if qt == 0 and kc == 1:
                                load_wp_slab(0)
                            elif qt == 0 and kc == 5:
                                load_wp_slab(1)
                            elif (qt, kc) in OP_SLOTS:
                                fc_ = OP_SLOTS[(qt, kc)]
                                emit_op_unit_alt(fc_, 0)
                                if fc_ + 2 < 10:
                                    load_wp_slab(fc_ + 2)
                            elif qt in (0, 1) and kc in (6, 12):
                                # no fillable work exists yet in h7's first
                                # two qts (scalar-bound); burn one junk MM
                                # so PE idle gaps stay under the ~3.4 us
                                # HAM re-throttle window
                                jp_ = ps_qkv.tile([128, 512], F32,
                                                  tag="psqkv",
                                                  name=f"junk7_{qt}_{kc}")
                                nc.tensor.matmul(jp_[:], jw[:, 0:128], jw[:],
                                                 start=True, stop=True)
                    pv(KC - 2)
                    pv(KC - 1)
                    # softmax denominator: f32 partition all-reduce on gpsimd
                    # (every partition gets the 512 per-token sums); the
                    # recip + (o * recip) normalize is deferred (see above).
                    eb = eb_pool.tile([128, 512], F32, tag="eb")
                    nc.gpsimd.partition_all_reduce(
                        eb[:], esum[:], 128, bass_isa.ReduceOp.add)
                    pending_norms.append((h, qt, o_ps, eb))

                # V(g1) block between head 3 and head 4: v_sb[0] reads are
                # all emitted by now, so the single vg buffer can recycle.
                if h == 3:
                    v_sb[1] = vg_pool.tile([128, KC, 512], BF16, tag="vg",
                                           name="vg1")
                    for tp in range(8):
                        emit_v_chunk(1, tp)

            # ------------- rest of the output projection -------------
            # tq0 tail (slabs 10..15 + second-round 0,1 prefetched 2 ahead);
            # the last pending norm (7,qt3) flushes once its all-reduce has
            # had ~5 us to finish
            for fc in range(8, CC):
                if fc + 2 < CC:
                    load_wp_slab(fc + 2)
                else:
                    load_wp_slab(fc + 2 - CC)
                emit_op_unit_alt(fc, 0)
                if fc == 10:
                    while pending_norms:
                        emit_norm(*pending_norms.pop(0))
            # tq 1..3 fc-major: one slab load per fc serves three units
            for fc in range(CC):
                for i, tq in enumerate((1, 2, 3)):
                    if i == 0 and fc + 2 < CC:
                        load_wp_slab(fc + 2)
                    emit_op_unit_alt(fc, tq)

    # populate .instr bytes for extended-inst InstISA subclasses
    # (partition_all_reduce) — raw Bass doesn't run this pass and the NEFF
    # compiler rejects empty .instr with "ISA wrong length"
    mybir.codegen_inst_isa_subclasses(nc)
    _split_multi_waits(nc)
    return nc


# ---------------------------------------------------------------------------
# host-side prep / assembly
# ---------------------------------------------------------------------------


def _to_bf16(a):
    return np.ascontiguousarray(a.astype(ml_dtypes.bfloat16))


def prep_inputs(x, w_attn, w_proj):
    x = np.asarray(x, dtype=np.float32)
    w_attn = np.asarray(w_attn, dtype=np.float32)
    w_proj = np.asarray(w_proj, dtype=np.float32)

    perm = np.concatenate([np.arange(0, HD, 2), np.arange(1, HD, 2)])

    inv = 1.0 / (10000.0 ** (np.arange(0, HD, 2, dtype=np.float64) / HD))
    fr = np.outer(np.arange(T, dtype=np.float64), inv)
    cos = np.cos(fr).T
    sin = np.sin(fr).T
    cs2 = _to_bf16(np.concatenate([cos, cos], 0))
    sn2 = _to_bf16(np.concatenate([-sin, sin], 0))

    # per head-half weight slabs (shared across batches)
    half_slabs = []
    for hh in range(2):
        heads = range(hh * NH, (hh + 1) * NH)
        # wq/wk: [NH, 128p(c within cc), CC*128f] with rope perm on f
        wq_sl = np.empty((NH, 128, CC * 128), dtype=np.float32)
        wk_sl = np.empty((NH, 128, CC * 128), dtype=np.float32)
        for jj, h in enumerate(heads):
            wq_h = w_attn[h * HD:(h + 1) * HD][perm, :]        # [128f, 2048c]
            wk_h = w_attn[D + h * HD:D + (h + 1) * HD][perm, :]
            # slab[p, cc, f] = w[f, cc*128+p]
            wq_sl[jj] = wq_h.T.reshape(CC, 128, 128).transpose(1, 0, 2).reshape(128, -1)
            wk_sl[jj] = wk_h.T.reshape(CC, 128, 128).transpose(1, 0, 2).reshape(128, -1)
        # wv: [2 groups, 2 halves, 128p, 8cc*(4h*128)]
        wv_sl = np.empty((2, 2, 128, 8 * 512), dtype=np.float32)
        for g in range(2):
            hv = w_attn[2 * D + (hh * NH + g * 4) * HD:
                        2 * D + (hh * NH + (g + 1) * 4) * HD]  # [512f, 2048c]
            # [cc, p, f] -> [half, 128p, 8cc, 512f]
            arr = hv.T.reshape(CC, 128, 512)
            for half in range(2):
                wv_sl[g, half] = (arr[half * 8:(half + 1) * 8]
                                  .transpose(1, 0, 2).reshape(128, -1))
        # wp: [128p(hd within hc), fc, hc, 128f]
        #   value = w_proj[fc*128+f, hh*1024 + hc*128 + p]
        wp_cols = w_proj[:, hh * NH * HD:(hh + 1) * NH * HD]  # [2048f, 1024hd]
        wp_sl = (wp_cols.T.reshape(NH, 128, CC, 128)
                 .transpose(1, 2, 0, 3))                       # [128p, fc, hc, f]
        half_slabs.append((_to_bf16(wq_sl), _to_bf16(wk_sl), _to_bf16(wv_sl),
                           _to_bf16(np.ascontiguousarray(wp_sl))))

    xTs = []
    for b in range(B):
        xT = x[b].T.reshape(CC, 128, T)
        xTs.append(_to_bf16(xT))

    in_maps = []
    for i in range(N_CORES):
        b, hh = i // 2, i % 2
        wq_sl, wk_sl, wv_sl, wp_sl = half_slabs[hh]
        in_maps.append({
            "xT": xTs[b],
            "wqs": wq_sl, "wks": wk_sl, "wvs": wv_sl, "wps": wp_sl,
            "cs2": cs2, "sn2": sn2,
        })
    return in_maps


def assemble(results):
    out = np.empty((B, T, D), dtype=np.float32)
    for b in range(B):
        p0 = results[2 * b]["out"].astype(np.float32)
        p1 = results[2 * b + 1]["out"].astype(np.float32)
        out[b] = (p0 + p1).T
    return out


_nc_cache = None


def _get_nc():
    global _nc_cache
    if _nc_cache is None:
        _nc_cache = build_nc()
    return _nc_cache


def kernel(x, w_attn, w_proj):
    from concourse.bass_utils import run_bass_kernel_spmd
    nc = _get_nc()
    in_maps = prep_inputs(x, w_attn, w_proj)
    res = run_bass_kernel_spmd(nc, in_maps, list(range(N_CORES)))
    return assemble(res.results)


def run_profiled(x, w_attn, w_proj, trace_cores=None):
    """Like kernel() but with NTFF profiling; returns BassKernelResults."""
    from concourse.bass_utils import run_bass_kernel_spmd
    import sys as _sys, types as _types
    try:
        import antenv
        if "antenv.axon_hooks" not in _sys.modules:
            mod = _types.ModuleType("antenv.axon_hooks")
            _h = [None]
            mod.set_axon_ntff_profile_hook = lambda h: _h.__setitem__(0, h)
            mod.get_axon_ntff_profile_hook = lambda: _h[0]
            _sys.modules["antenv.axon_hooks"] = mod
            antenv.axon_hooks = mod
            from trn_agent_boot.trn_boot import _ntff_profile_via_ctypes
            mod.set_axon_ntff_profile_hook(
                _ntff_profile_via_ctypes('/opt/axon/libaxon_pjrt.so'))
    except Exception as e:  # profiling is best-effort
        print("profile hook setup failed:", e)
    nc = _get_nc()
    in_maps = prep_inputs(x, w_attn, w_proj)
    return run_bass_kernel_spmd(
        nc, in_maps, list(range(N_CORES)), trace=True,
        trace_cores=trace_cores if trace_cores is not None else [0])


# revision 39
# speedup vs baseline: 1.3044x; 1.0139x over previous
"""8-core Trainium2 Bass kernel v3 for nn_Attention_89489938579587.

reference: qkv = x @ w_attn.T; split q,k,v per 16 heads (HD=128); RoPE
(interleaved pairs); non-causal SDPA; y @ w_proj.T.  B=4, T=2048, D=2048.

Sharding: core i -> (batch b=i//2, head-half hh=i%2).  Each core computes
QKV for its 8 heads over ALL 2048 tokens, RoPE, SDPA, and a PARTIAL output
projection (contraction over its 1024 head-dims) in f-major layout
[2048 f, 2048 t].  Host adds the two partials per batch and transposes.
Ideal 1/8 compute share (25.8 GMAC/core = 3072 N=512 matmuls); no
collectives.

v3 changes over v2 (778 us):
 - softmax denominator reduce + broadcast moved off the PE: gpsimd
   partition_all_reduce + DVE reciprocal + fused scalar_tensor_tensor
   normalize (removes 64 aux/bc matmuls = ~22 us PE busy + disruption).
 - output projection restarted tq-major and started EARLY: 8 units
   interleaved into head-7's SDPA (PE filler while the exp chain drains;
   kills the 2.5-2.9 us/qt stalls + HAM 4/8 oscillation seen in the v2
   trace tail).  w_proj fully SBUF-resident after x is freed at h3.
 - bootstrap: x DMA'd in 512-token blocks with weight DMAs front-loaded
   so the first matmul starts ~7 us (was 16.8); junk matmuls on a memset
   tile pre-warm the HAM clock gate; dummy exp preloads the act table.
 - producer psum->sbuf copies moved from scalar to vector so the scalar
   engine only runs exp (it was the binding engine in the tail).
"""

import numpy as np
from contextlib import ExitStack

import concourse.bass as bass
import concourse.tile as tile
from concourse import mybir
from concourse import bass_isa
from concourse import library_config
from concourse.bass import ts

import bass_rust
import ml_dtypes

# ---------------------------------------------------------------------------
# Toolchain workarounds (same as baseline): walrus rejects >1 sem wait per
# instruction; split extras onto same-engine nops; patch tile drain.
# ---------------------------------------------------------------------------


def _split_multi_waits(nc, max_waits=1):
    n = 0
    for fn in nc.m.functions:
        for blk in fn.blocks:
            insts = blk.instructions
            i = 0
            while i < len(insts):
                inst = insts[i]
                si = inst.sync_info
                waits = list(si.on_wait) if (si is not None and si.on_wait) else []
                if len(waits) > max_waits:
                    si.on_wait = waits[:max_waits]
                    extra = waits[max_waits:]
                    for j in range(0, len(extra), max_waits):
                        nop = mybir.InstNoOp(
                            name=nc.get_next_instruction_name(), ins=[], outs=[])
                        nop.engine = inst.engine
                        nop.sync_info = bass_rust.SyncInfo(
                            on_wait=extra[j:j + max_waits], on_update=[])
                        nc.register_instruction(nop, overwrite=True)
                        insts.insert(i, nop)
                        i += 1
                        n += 1
                i += 1
    return n


def _patched_drain_and_barrier(self, tick_clock, wait_clock):
    from concourse.vector_clock import ScopedClock
    nc = self.nc
    probe = nc.sync.nop()
    wait_clock.add_sem_waits(probe.ins, ScopedClock({None: tick_clock.global_clock}))
    si = probe.ins.sync_info
    waits = list(si.on_wait or []) if si is not None else []
    if len(waits) > 1:
        si.on_wait = [waits[0]]
        for w in waits[1:]:
            nop = nc.sync.nop()
            nsi = nop.ins.sync_info
            if nsi is None:
                nop.ins.sync_info = bass_rust.SyncInfo(on_wait=[w], on_update=[])
            else:
                nsi.on_wait = [w]
    nc.sync.drain()
    nc.all_engine_barrier()
    assert self.sems is not None
    popped = nc._tile_sem_poison_stack.pop()
    assert popped is self._sem_poison
    nc.clear_and_free_semaphores(list(self.sems.allocated().values()))
    nc.all_engine_barrier()


_patched = False


def _apply_patches():
    global _patched
    if not _patched:
        tile.TileContext._drain_and_barrier = _patched_drain_and_barrier
        _patched = True


# ---------------------------------------------------------------------------
# Problem constants
# ---------------------------------------------------------------------------
BF16 = mybir.dt.bfloat16
F32 = mybir.dt.float32
EXP = mybir.ActivationFunctionType.Exp
MULT = mybir.AluOpType.mult

B, T, D, H, HD = 4, 2048, 2048, 16, 128
CC = D // 128            # 16 contraction chunks
NH = 8                   # heads per core
KC = T // 128            # 16 key chunks
NB = 4                   # 512-token x blocks
SCALE = 1.0 / float(np.sqrt(HD))
N_CORES = 8

# out-proj units interleaved into head-7's SDPA: (qt, kc) -> fc of tq0
OP_SLOTS = {(2, 1): 0, (2, 5): 1, (2, 9): 2, (2, 13): 3,
            (3, 1): 4, (3, 5): 5, (3, 9): 6, (3, 13): 7}


def build_nc(n_cores=N_CORES):
    _apply_patches()
    nc = bass.Bass("TRN2", target_bir_lowering=False, debug=False,
                   num_devices=n_cores)
    xT = nc.dram_tensor("xT", [CC, 128, T], BF16, kind="ExternalInput").ap()
    wqs = nc.dram_tensor("wqs", [NH, 128, CC * 128], BF16, kind="ExternalInput").ap()
    wks = nc.dram_tensor("wks", [NH, 128, CC * 128], BF16, kind="ExternalInput").ap()
    # V weights: [group, cc-half, 128p, 8cc * (4h*128f)]
    wvs = nc.dram_tensor("wvs", [2, 2, 128, 8 * 512], BF16, kind="ExternalInput").ap()
    # out-proj: [128p(hd), fc, hc, 128f]
    wps = nc.dram_tensor("wps", [128, CC, NH, 128], BF16, kind="ExternalInput").ap()
    cs2 = nc.dram_tensor("cs2", [128, T], BF16, kind="ExternalInput").ap()
    sn2 = nc.dram_tensor("sn2", [128, T], BF16, kind="ExternalInput").ap()
    # f-major partial output [2048 f, 2048 t]
    out = nc.dram_tensor("out", [D, T], BF16, kind="ExternalOutput").ap()

    with tile.TileContext(nc) as tc, ExitStack() as octx:
        # gpsimd ucode library with partition_all_reduce (loads at t=0,
        # first use is ~60 us in)
        nc.gpsimd.load_library(library_config.attn)

        cs_pool = octx.enter_context(tc.tile_pool(name="cs", bufs=1))
        csk = cs_pool.tile([128, T], BF16, tag="csk")
        snk = cs_pool.tile([128, T], BF16, tag="snk")
        jw = cs_pool.tile([128, 512], BF16, tag="jw")
        dexp = cs_pool.tile([1, 16], BF16, tag="dexp")

        oT_pool = octx.enter_context(tc.tile_pool(name="oT", bufs=1))
        oT = oT_pool.tile([128, NH, T], BF16, tag="oT")

        with ExitStack() as p1:
            wqk_pool = p1.enter_context(tc.tile_pool(name="wqk", bufs=4))
            wv_pool = p1.enter_context(tc.tile_pool(name="wv", bufs=2))
            vg_pool = p1.enter_context(tc.tile_pool(name="vg", bufs=1))
            q_pool = p1.enter_context(tc.tile_pool(name="q", bufs=2))
            k_pool = p1.enter_context(tc.tile_pool(name="k", bufs=2))
            rp_pool = p1.enter_context(tc.tile_pool(name="rp", bufs=2))
            eT_pool = p1.enter_context(tc.tile_pool(name="eT", bufs=8))
            es_pool = p1.enter_context(tc.tile_pool(name="es", bufs=2))
            eb_pool = p1.enter_context(tc.tile_pool(name="eb", bufs=2))
            rb_pool = p1.enter_context(tc.tile_pool(name="rb", bufs=2))
            ob_pool = p1.enter_context(tc.tile_pool(name="ob", bufs=4))
            wpf_pool = p1.enter_context(tc.tile_pool(name="wpf", bufs=4))
            # 8 PSUM banks: qkv/out-proj units share one pool (producers are
            # done before the out-proj starts); ps_o double-buffered so the
            # slow gpsimd norm chain reads bank A while qt+1 accumulates in
            # bank B (chain gets ~28 us of slack, PE never waits on it)
            ps_qkv = p1.enter_context(tc.tile_pool(name="psqkv", bufs=2, space="PSUM"))
            ps_s = p1.enter_context(tc.tile_pool(name="pss", bufs=4, space="PSUM"))
            ps_o = p1.enter_context(tc.tile_pool(name="pso", bufs=2, space="PSUM"))

            x_pool = p1.enter_context(tc.tile_pool(name="x", bufs=1))

            # x resident: 16 [128, 2048] tiles.  One dma_start per cc tile —
            # the sync engine issues DMAs serially at ~650 ns each, so fewer,
            # bigger issues beat fine-grained blocks (v4 measured 57 GB/s
            # effective with 64 block DMAs: issue-bound, queues 7% busy).
            xs = [None] * CC

            # per-core state holders
            q_sb = [None] * NH
            k_sb = [None] * NH
            v_sb = [None] * 2   # per group
            v_w = [[None, None], [None, None]]
            w_hold = {}
            sf_hold = {}
            wp_slabs = {}

            def load_wp_slab(fc):
                w_ = wpf_pool.tile([128, NH, 128], BF16, tag="wpf",
                                   name=f"wpf{fc}")
                nc.sync.dma_start(w_[:], wps[:, fc])
                wp_slabs[fc] = w_

            def load_vw(g, half):
                w_ = wv_pool.tile([128, 8 * 512], BF16, tag="wv")
                nc.sync.dma_start(w_[:], wvs[g, half])
                v_w[g][half] = w_

            def ensure_wqk(kind, h):
                if (kind, h) not in w_hold:
                    wsl = wqk_pool.tile([128, CC, 128], BF16, tag="wqk")
                    nc.sync.dma_start(wsl[:], (wqs if kind == "q" else wks)[h])
                    w_hold[(kind, h)] = wsl
                return w_hold[(kind, h)]

            def emit_proj_half(kind, h, tpair, half):
                """One 16-matmul unit: 512 tokens of a Q/K projection.  After
                the second half of a token-pair, the RoPE chain is emitted."""
                wsl = ensure_wqk(kind, h)
                toff = tpair * 1024
                if half == 0:
                    sf_hold[kind] = rp_pool.tile([128, 1024], BF16, tag="sf",
                                                 name=f"sf_{kind}{h}_{tpair}")
                sf = sf_hold[kind]
                ps = ps_qkv.tile([128, 512], F32, tag="psqkv")
                for cc in range(CC):
                    nc.tensor.matmul(
                        ps[:], wsl[:, cc, :],
                        xs[cc][:, toff + half * 512: toff + (half + 1) * 512],
                        start=(cc == 0), stop=(cc == CC - 1))
                # scalar copy: keeps the psum->sbuf copy out of the DVE FIFO
                # where it sat behind RoPE ops and gated the unit 2 later
                nc.scalar.copy(sf[:, ts(half, 512)], ps[:])
                if half == 0:
                    return
                sw = rp_pool.tile([128, 1024], BF16, tag="sw")
                nc.sync.dma_start(sw[0:64, :], sf[64:128, :])
                nc.sync.dma_start(sw[64:128, :], sf[0:64, :])
                nc.vector.tensor_mul(sf[:], sf[:], csk[:, toff:toff + 1024])
                nc.vector.tensor_mul(sw[:], sw[:], snk[:, toff:toff + 1024])
                dst = q_sb[h] if kind == "q" else k_sb[h]
                nc.vector.tensor_add(dst[:, toff:toff + 1024], sf[:], sw[:])

            def producer_units(hn):
                """Generator of head-hn QKV producer units (8 per head)."""
                q_sb[hn] = q_pool.tile([128, T], BF16, tag="q", name=f"qh{hn}")
                k_sb[hn] = k_pool.tile([128, T], BF16, tag="k", name=f"kh{hn}")
                for kind in ("q", "k"):
                    for tpair in range(2):
                        for half in range(2):
                            yield (kind, hn, tpair, half)

            def emit_v_chunk(g, tch_pair):
                """V for head-group g, two token chunks (2*128 tokens)."""
                for u in range(2):
                    tch = tch_pair * 2 + u
                    ps = ps_qkv.tile([128, 512], F32, tag="psqkv")
                    for cc in range(CC):
                        wv_ap = v_w[g][cc // 8][:, (cc % 8) * 512:(cc % 8 + 1) * 512]
                        nc.tensor.matmul(
                            ps[:], xs[cc][:, ts(tch, 128)], wv_ap,
                            start=(cc == 0), stop=(cc == CC - 1))
                    if tch % 2 == 0:
                        nc.scalar.copy(v_sb[g][:, tch, :], ps[:])
                    else:
                        nc.vector.tensor_copy(v_sb[g][:, tch, :], ps[:])

            op_state = {"alt": 0}

            def emit_op_unit(fc, tq, eng):
                """Partial out-proj: one [128f x 512t] tile, contraction over
                this core's 8 heads."""
                wp = wp_slabs[fc]
                ps = ps_qkv.tile([128, 512], F32, tag="psqkv")
                for hc in range(NH):
                    nc.tensor.matmul(ps[:], wp[:, hc, :],
                                     oT[:, hc, ts(tq, 512)],
                                     start=(hc == 0), stop=(hc == NH - 1))
                oe = ob_pool.tile([128, 512], BF16, tag="ob")
                if eng == "v":
                    nc.vector.tensor_copy(oe[:], ps[:])
                else:
                    nc.scalar.copy(oe[:], ps[:])
                nc.sync.dma_start(out[ts(fc, 128), ts(tq, 512)], oe[:])

            def emit_op_unit_alt(fc, tq):
                op_state["alt"] ^= 1
                emit_op_unit(fc, tq, "s" if op_state["alt"] else "v")

            # ---------------- bootstrap ----------------
            # HAM warm-up: junk matmuls on a memset tile while x DMA lands.
            nc.vector.memset(jw[:], 0.0)
            jp = ps_qkv.tile([128, 512], F32, tag="psqkv", name="junk")
            for i in range(14):
                nc.tensor.matmul(jp[:], jw[:, 0:128], jw[:],
                                 start=True, stop=True)

            # DMA order = sync-queue order.  The sync engine issues DMAs
            # serially at ~1.3 us each, so small high-priority loads go
            # first; wv(1,*) reuse slots whose WAR wait would BLOCK the
            # queue, so they are emitted after the bootstrap (see below).
            # spread the 22 bootstrap DMA issues over both HWDGE-capable
            # queues — sync (SP) and scalar (Activation) — each issues
            # serially at ~1.3 us/DMA
            for cc in range(CC):
                t_ = x_pool.tile([128, T], BF16, tag=f"x{cc}", name=f"x{cc}")
                xs[cc] = t_
            load_vw(0, 0)
            nc.sync.dma_start(xs[0][:], xT[0])
            nc.sync.dma_start(xs[1][:], xT[1])
            ensure_wqk("q", 0)
            ensure_wqk("k", 0)
            nc.sync.dma_start(csk[:], cs2[:])
            nc.sync.dma_start(snk[:], sn2[:])
            for cc in range(2, 8):
                nc.sync.dma_start(xs[cc][:], xT[cc])
            w_ = wv_pool.tile([128, 8 * 512], BF16, tag="wv", name="wv01")
            nc.scalar.dma_start(w_[:], wvs[0, 1])
            v_w[0][1] = w_
            for cc in range(8, CC):
                nc.scalar.dma_start(xs[cc][:], xT[cc])
            # dummy exp after the scalar-queue issues: walrus puts the
            # ~2.7 us act table load here, off the critical path
            nc.scalar.activation(dexp[:], csk[0:1, 0:16], EXP, scale=1.0)

            # cc-outer "wave" emission: each issuing queue serializes its
            # DMA transfers (~170 GB/s/queue), so x tiles arrive one-by-one
            # over ~25 us.  8 PSUM accumulators walked cc-outer consume each
            # xs[cc] for ~1.7 us the moment it lands instead of stalling a
            # whole 16-MM chain on the last cc.
            v_sb[0] = vg_pool.tile([128, KC, 512], BF16, tag="vg", name="vg0")
            q_sb[0] = q_pool.tile([128, T], BF16, tag="q", name="qh0")
            k_sb[0] = k_pool.tile([128, T], BF16, tag="k", name="kh0")

            def wave_psums(nm):
                return ([ps_qkv.tile([128, 512], F32, tag="psqkv",
                                     name=f"{nm}{i}") for i in range(2)]
                        + [ps_s.tile([128, 512], F32, tag="pss",
                                     name=f"{nm}{i + 2}") for i in range(4)]
                        + [ps_o.tile([128, 512], F32, tag="pso",
                                     name=f"{nm}{i + 6}") for i in range(2)])

            def v_wave(g, base):
                wt = wave_psums(f"wv{g}_{base}_")
                for cc in range(CC):
                    wv_ap = v_w[g][cc // 8][:, (cc % 8) * 512:(cc % 8 + 1) * 512]
                    for i in range(8):
                        nc.tensor.matmul(
                            wt[i][:], xs[cc][:, ts(base + i, 128)], wv_ap,
                            start=(cc == 0), stop=(cc == CC - 1))
                for i in range(8):
                    if i % 2 == 0:
                        nc.scalar.copy(v_sb[g][:, base + i, :], wt[i][:])
                    else:
                        nc.vector.tensor_copy(v_sb[g][:, base + i, :], wt[i][:])

            def qk_wave(h):
                units = [(kind, tpair, half) for kind in ("q", "k")
                         for tpair in range(2) for half in range(2)]
                wt = wave_psums(f"wqk{h}_")
                for cc in range(CC):
                    for i, (kind, tpair, half) in enumerate(units):
                        toff = tpair * 1024 + half * 512
                        nc.tensor.matmul(
                            wt[i][:], w_hold[(kind, h)][:, cc, :],
                            xs[cc][:, toff:toff + 512],
                            start=(cc == 0), stop=(cc == CC - 1))
                for kind in ("q", "k"):
                    for tpair in range(2):
                        i0 = (0 if kind == "q" else 4) + tpair * 2
                        toff = tpair * 1024
                        sf = rp_pool.tile([128, 1024], BF16, tag="sf",
                                          name=f"sfw_{kind}{tpair}")
                        nc.scalar.copy(sf[:, 0:512], wt[i0][:])
                        nc.scalar.copy(sf[:, 512:1024], wt[i0 + 1][:])
                        sw = rp_pool.tile([128, 1024], BF16, tag="sw")
                        nc.sync.dma_start(sw[0:64, :], sf[64:128, :])
                        nc.sync.dma_start(sw[64:128, :], sf[0:64, :])
                        nc.vector.tensor_mul(sf[:], sf[:], csk[:, toff:toff + 1024])
                        nc.vector.tensor_mul(sw[:], sw[:], snk[:, toff:toff + 1024])
                        dst = q_sb[h] if kind == "q" else k_sb[h]
                        nc.vector.tensor_add(dst[:, toff:toff + 1024], sf[:], sw[:])

            v_wave(0, 0)
            qk_wave(0)
            v_wave(0, 8)
            # h1 slabs (fresh wqk slots, no WAR wait) then the g1 V weights,
            # whose slot-reuse WAR wait blocks the sync queue until the g0
            # V matmuls have all read their slabs — nothing urgent behind.
            ensure_wqk("q", 1)
            ensure_wqk("k", 1)
            load_vw(1, 0)
            load_vw(1, 1)

            # Deferred softmax normalization: the gpsimd all-reduce takes
            # ~3.6 us, and the DVE queue is strict FIFO — an immediately-
            # emitted reciprocal would block the next qt's esum adds behind
            # the all-reduce (measured 4.9 us PE gap per qt).  Defer the
            # recip+normalize to kc==10 of the NEXT qt, when the all-reduce
            # result is long ready.
            pending_norms = []

            def emit_norm(h_, qt_, o_ps_t, eb_t):
                # approx recip: ~0.7 us vs 3.35 us for nc.vector.reciprocal,
                # ~51 ULP f32 — far better than the bf16 denom it replaces
                rb = rb_pool.tile([128, 512], F32, tag="rb")
                nc.vector.reciprocal_approx_fast(out=rb[:], in_=eb_t[:])
                with nc.allow_low_precision(reason="bf16 attn out"):
                    nc.vector.scalar_tensor_tensor(
                        oT[:, h_, ts(qt_, 512)], o_ps_t[:], 1.0, rb[:],
                        MULT, MULT)

            def pop_norm(lag=0):
                if len(pending_norms) > lag:
                    emit_norm(*pending_norms.pop(0))

            # ---------------- main loop: SDPA per head ----------------
            for h in range(NH):
                g, j = h // 4, h % 4
                prod = producer_units(h + 1) if h + 1 < NH else iter(())
                if h + 1 < NH:
                    # prefetch next head's Q/K slabs so the kc==1 producer
                    # unit never waits on the ~1.3 us/DMA sync issue queue
                    ensure_wqk("q", h + 1)
                    ensure_wqk("k", h + 1)
                for qt in range(4):
                    qsl = q_sb[h][:, ts(qt, 512)]
                    esum = es_pool.tile([128, 512], BF16, tag="es")
                    o_ps = ps_o.tile([128, 512], F32, tag="pso")
                    eTs = [None] * KC

                    def pv(kc):
                        nc.tensor.matmul(
                            o_ps[:], v_sb[g][:, kc, ts(j, 128)], eTs[kc][:],
                            start=(kc == 0), stop=(kc == KC - 1))

                    for kc in range(KC):
                        s_ps = ps_s.tile([128, 512], F32, tag="pss")
                        nc.tensor.matmul(s_ps[:], k_sb[h][:, ts(kc, 128)], qsl,
                                         start=True, stop=True)
                        eT = eT_pool.tile([128, 512], BF16, tag="eT")
                        nc.scalar.activation(eT[:], s_ps[:], EXP, scale=SCALE)
                        eTs[kc] = eT
                        # bf16 chunk-sum (magnitude ~30; the 2048-wide key
                        # reduction happens exactly in f32 on gpsimd below)
                        with nc.allow_low_precision(reason="bf16 chunk sum"):
                            if kc == 0:
                                nc.vector.tensor_copy(esum[:], eT[:])
                            else:
                                nc.vector.tensor_add(esum[:], esum[:], eT[:])
                        if kc >= 2:
                            pv(kc - 2)
                        if kc == 12:
                            pop_norm()
                        if h < NH - 1 and kc in (1, 7):
                            # slots early in the qt so the last unit's RoPE
                            # chain lands before the next head's first scores
                            unit = next(prod, None)
                            if unit is not None:
                                emit_proj_half(*unit)
                        if h == NH - 1:
                            if qt == 0 and kc == 1:
                                load_wp_slab(0)
                            elif qt == 0 and kc == 5:
                                load_wp_slab(1)
                            elif (qt, kc) in OP_SLOTS:
                                fc_ = OP_SLOTS[(qt, kc)]
                                emit_op_unit_alt(fc_, 0)
                                if fc_ + 2 < 10:
                                    load_wp_slab(fc_ + 2)
                            elif qt in (0, 1) and kc in (4, 6, 8, 10, 14):
                                # no fillable work exists yet in h7's first
                                # two qts (scalar-bound, ~2.8 us PE deficit
                                # per qt); burn junk MMs so PE idle gaps
                                # stay under the ~3.4 us HAM window
                                jp_ = ps_qkv.tile([128, 512], F32,
                                                  tag="psqkv",
                                                  name=f"junk7_{qt}_{kc}")
                                nc.tensor.matmul(jp_[:], jw[:, 0:128], jw[:],
                                                 start=True, stop=True)
                    pv(KC - 2)
                    pv(KC - 1)
                    # softmax denominator: f32 partition all-reduce on gpsimd
                    # (every partition gets the 512 per-token sums); the
                    # recip + (o * recip) normalize is deferred (see above).
                    eb = eb_pool.tile([128, 512], F32, tag="eb")
                    nc.gpsimd.partition_all_reduce(
                        eb[:], esum[:], 128, bass_isa.ReduceOp.add)
                    pending_norms.append((h, qt, o_ps, eb))

                # V(g1) block between head 3 and head 4: v_sb[0] reads are
                # all emitted by now, so the single vg buffer can recycle.
                if h == 3:
                    v_sb[1] = vg_pool.tile([128, KC, 512], BF16, tag="vg",
                                           name="vg1")
                    for tp in range(8):
                        emit_v_chunk(1, tp)

            # ------------- rest of the output projection -------------
            # tq0 tail (slabs 10..15 + second-round 0,1 prefetched 2 ahead);
            # the last pending norm (7,qt3) flushes once its all-reduce has
            # had ~5 us to finish
            for fc in range(8, CC):
                if fc + 2 < CC:
                    load_wp_slab(fc + 2)
                else:
                    load_wp_slab(fc + 2 - CC)
                emit_op_unit_alt(fc, 0)
                if fc == 10:
                    while pending_norms:
                        emit_norm(*pending_norms.pop(0))
            # tq 1..3 fc-major: one slab load per fc serves three units
            for fc in range(CC):
                for i, tq in enumerate((1, 2, 3)):
                    if i == 0 and fc + 2 < CC:
                        load_wp_slab(fc + 2)
                    emit_op_unit_alt(fc, tq)

    # populate .instr bytes for extended-inst InstISA subclasses
    # (partition_all_reduce) — raw Bass doesn't run this pass and the NEFF
    # compiler rejects empty .instr with "ISA wrong length"
    mybir.codegen_inst_isa_subclasses(nc)
    _split_multi_waits(nc)
    return nc


# ---------------------------------------------------------------------------
# host-side prep / assembly
# ---------------------------------------------------------------------------


def _to_bf16(a):
    return np.ascontiguousarray(a.astype(ml_dtypes.bfloat16))


def prep_inputs(x, w_attn, w_proj):
    x = np.asarray(x, dtype=np.float32)
    w_attn = np.asarray(w_attn, dtype=np.float32)
    w_proj = np.asarray(w_proj, dtype=np.float32)

    perm = np.concatenate([np.arange(0, HD, 2), np.arange(1, HD, 2)])

    inv = 1.0 / (10000.0 ** (np.arange(0, HD, 2, dtype=np.float64) / HD))
    fr = np.outer(np.arange(T, dtype=np.float64), inv)
    cos = np.cos(fr).T
    sin = np.sin(fr).T
    cs2 = _to_bf16(np.concatenate([cos, cos], 0))
    sn2 = _to_bf16(np.concatenate([-sin, sin], 0))

    # per head-half weight slabs (shared across batches)
    half_slabs = []
    for hh in range(2):
        heads = range(hh * NH, (hh + 1) * NH)
        # wq/wk: [NH, 128p(c within cc), CC*128f] with rope perm on f
        wq_sl = np.empty((NH, 128, CC * 128), dtype=np.float32)
        wk_sl = np.empty((NH, 128, CC * 128), dtype=np.float32)
        for jj, h in enumerate(heads):
            wq_h = w_attn[h * HD:(h + 1) * HD][perm, :]        # [128f, 2048c]
            wk_h = w_attn[D + h * HD:D + (h + 1) * HD][perm, :]
            # slab[p, cc, f] = w[f, cc*128+p]
            wq_sl[jj] = wq_h.T.reshape(CC, 128, 128).transpose(1, 0, 2).reshape(128, -1)
            wk_sl[jj] = wk_h.T.reshape(CC, 128, 128).transpose(1, 0, 2).reshape(128, -1)
        # wv: [2 groups, 2 halves, 128p, 8cc*(4h*128)]
        wv_sl = np.empty((2, 2, 128, 8 * 512), dtype=np.float32)
        for g in range(2):
            hv = w_attn[2 * D + (hh * NH + g * 4) * HD:
                        2 * D + (hh * NH + (g + 1) * 4) * HD]  # [512f, 2048c]
            # [cc, p, f] -> [half, 128p, 8cc, 512f]
            arr = hv.T.reshape(CC, 128, 512)
            for half in range(2):
                wv_sl[g, half] = (arr[half * 8:(half + 1) * 8]
                                  .transpose(1, 0, 2).reshape(128, -1))
        # wp: [128p(hd within hc), fc, hc, 128f]
        #   value = w_proj[fc*128+f, hh*1024 + hc*128 + p]
        wp_cols = w_proj[:, hh * NH * HD:(hh + 1) * NH * HD]  # [2048f, 1024hd]
        wp_sl = (wp_cols.T.reshape(NH, 128, CC, 128)
                 .transpose(1, 2, 0, 3))                       # [128p, fc, hc, f]
        half_slabs.append((_to_bf16(wq_sl), _to_bf16(wk_sl), _to_bf16(wv_sl),
                           _to_bf16(np.ascontiguousarray(wp_sl))))

    xTs = []
    for b in range(B):
        xT = x[b].T.reshape(CC, 128, T)
        xTs.append(_to_bf16(xT))

    in_maps = []
    for i in range(N_CORES):
        b, hh = i // 2, i % 2
        wq_sl, wk_sl, wv_sl, wp_sl = half_slabs[hh]
        in_maps.append({
            "xT": xTs[b],
            "wqs": wq_sl, "wks": wk_sl, "wvs": wv_sl, "wps": wp_sl,
            "cs2": cs2, "sn2": sn2,
        })
    return in_maps


def assemble(results):
    out = np.empty((B, T, D), dtype=np.float32)
    for b in range(B):
        p0 = results[2 * b]["out"].astype(np.float32)
        p1 = results[2 * b + 1]["out"].astype(np.float32)
        out[b] = (p0 + p1).T
    return out


_nc_cache = None


def _get_nc():
    global _nc_cache
    if _nc_cache is None:
        _nc_cache = build_nc()
    return _nc_cache


def kernel(x, w_attn, w_proj):
    from concourse.bass_utils import run_bass_kernel_spmd
    nc = _get_nc()
    in_maps = prep_inputs(x, w_attn, w_proj)
    res = run_bass_kernel_spmd(nc, in_maps, list(range(N_CORES)))
    return assemble(res.results)


def run_profiled(x, w_attn, w_proj, trace_cores=None):
    """Like kernel() but with NTFF profiling; returns BassKernelResults."""
    from concourse.bass_utils import run_bass_kernel_spmd
    import sys as _sys, types as _types
    try:
        import antenv
        if "antenv.axon_hooks" not in _sys.modules:
            mod = _types.ModuleType("antenv.axon_hooks")
            _h = [None]
            mod.set_axon_ntff_profile_hook = lambda h: _h.__setitem__(0, h)
            mod.get_axon_ntff_profile_hook = lambda: _h[0]
            _sys.modules["antenv.axon_hooks"] = mod
            antenv.axon_hooks = mod
            from trn_agent_boot.trn_boot import _ntff_profile_via_ctypes
            mod.set_axon_ntff_profile_hook(
                _ntff_profile_via_ctypes('/opt/axon/libaxon_pjrt.so'))
    except Exception as e:  # profiling is best-effort
        print("profile hook setup failed:", e)
    nc = _get_nc()
    in_maps = prep_inputs(x, w_attn, w_proj)
    return run_bass_kernel_spmd(
        nc, in_maps, list(range(N_CORES)), trace=True,
        trace_cores=trace_cores if trace_cores is not None else [0])


# revision 40
# speedup vs baseline: 1.3351x; 1.0235x over previous
"""8-core Trainium2 Bass kernel v3 for nn_Attention_89489938579587.

reference: qkv = x @ w_attn.T; split q,k,v per 16 heads (HD=128); RoPE
(interleaved pairs); non-causal SDPA; y @ w_proj.T.  B=4, T=2048, D=2048.

Sharding: core i -> (batch b=i//2, head-half hh=i%2).  Each core computes
QKV for its 8 heads over ALL 2048 tokens, RoPE, SDPA, and a PARTIAL output
projection (contraction over its 1024 head-dims) in f-major layout
[2048 f, 2048 t].  Host adds the two partials per batch and transposes.
Ideal 1/8 compute share (25.8 GMAC/core = 3072 N=512 matmuls); no
collectives.

v3 changes over v2 (778 us):
 - softmax denominator reduce + broadcast moved off the PE: gpsimd
   partition_all_reduce + DVE reciprocal + fused scalar_tensor_tensor
   normalize (removes 64 aux/bc matmuls = ~22 us PE busy + disruption).
 - output projection restarted tq-major and started EARLY: 8 units
   interleaved into head-7's SDPA (PE filler while the exp chain drains;
   kills the 2.5-2.9 us/qt stalls + HAM 4/8 oscillation seen in the v2
   trace tail).  w_proj fully SBUF-resident after x is freed at h3.
 - bootstrap: x DMA'd in 512-token blocks with weight DMAs front-loaded
   so the first matmul starts ~7 us (was 16.8); junk matmuls on a memset
   tile pre-warm the HAM clock gate; dummy exp preloads the act table.
 - producer psum->sbuf copies moved from scalar to vector so the scalar
   engine only runs exp (it was the binding engine in the tail).
"""

import numpy as np
from contextlib import ExitStack

import concourse.bass as bass
import concourse.tile as tile
from concourse import mybir
from concourse import bass_isa
from concourse import library_config
from concourse.bass import ts

import bass_rust
import ml_dtypes

# ---------------------------------------------------------------------------
# Toolchain workarounds (same as baseline): walrus rejects >1 sem wait per
# instruction; split extras onto same-engine nops; patch tile drain.
# ---------------------------------------------------------------------------


def _split_multi_waits(nc, max_waits=1):
    n = 0
    for fn in nc.m.functions:
        for blk in fn.blocks:
            insts = blk.instructions
            i = 0
            while i < len(insts):
                inst = insts[i]
                si = inst.sync_info
                waits = list(si.on_wait) if (si is not None and si.on_wait) else []
                if len(waits) > max_waits:
                    si.on_wait = waits[:max_waits]
                    extra = waits[max_waits:]
                    for j in range(0, len(extra), max_waits):
                        nop = mybir.InstNoOp(
                            name=nc.get_next_instruction_name(), ins=[], outs=[])
                        nop.engine = inst.engine
                        nop.sync_info = bass_rust.SyncInfo(
                            on_wait=extra[j:j + max_waits], on_update=[])
                        nc.register_instruction(nop, overwrite=True)
                        insts.insert(i, nop)
                        i += 1
                        n += 1
                i += 1
    return n


def _patched_drain_and_barrier(self, tick_clock, wait_clock):
    from concourse.vector_clock import ScopedClock
    nc = self.nc
    probe = nc.sync.nop()
    wait_clock.add_sem_waits(probe.ins, ScopedClock({None: tick_clock.global_clock}))
    si = probe.ins.sync_info
    waits = list(si.on_wait or []) if si is not None else []
    if len(waits) > 1:
        si.on_wait = [waits[0]]
        for w in waits[1:]:
            nop = nc.sync.nop()
            nsi = nop.ins.sync_info
            if nsi is None:
                nop.ins.sync_info = bass_rust.SyncInfo(on_wait=[w], on_update=[])
            else:
                nsi.on_wait = [w]
    nc.sync.drain()
    nc.all_engine_barrier()
    assert self.sems is not None
    popped = nc._tile_sem_poison_stack.pop()
    assert popped is self._sem_poison
    nc.clear_and_free_semaphores(list(self.sems.allocated().values()))
    nc.all_engine_barrier()


_patched = False


def _apply_patches():
    global _patched
    if not _patched:
        tile.TileContext._drain_and_barrier = _patched_drain_and_barrier
        _patched = True


# ---------------------------------------------------------------------------
# Problem constants
# ---------------------------------------------------------------------------
BF16 = mybir.dt.bfloat16
F32 = mybir.dt.float32
EXP = mybir.ActivationFunctionType.Exp
MULT = mybir.AluOpType.mult

B, T, D, H, HD = 4, 2048, 2048, 16, 128
CC = D // 128            # 16 contraction chunks
NH = 8                   # heads per core
KC = T // 128            # 16 key chunks
NB = 4                   # 512-token x blocks
SCALE = 1.0 / float(np.sqrt(HD))
N_CORES = 8

# out-proj units interleaved into head-7's SDPA: (qt, kc) -> fc of tq0
OP_SLOTS = {(2, 1): 0, (2, 5): 1, (2, 9): 2, (2, 13): 3,
            (3, 1): 4, (3, 5): 5, (3, 9): 6, (3, 13): 7}


def build_nc(n_cores=N_CORES):
    _apply_patches()
    nc = bass.Bass("TRN2", target_bir_lowering=False, debug=False,
                   num_devices=n_cores)
    xT = nc.dram_tensor("xT", [CC, 128, T], BF16, kind="ExternalInput").ap()
    wqs = nc.dram_tensor("wqs", [NH, 128, CC * 128], BF16, kind="ExternalInput").ap()
    wks = nc.dram_tensor("wks", [NH, 128, CC * 128], BF16, kind="ExternalInput").ap()
    # V weights: [group, cc-half, 128p, 8cc * (4h*128f)]
    wvs = nc.dram_tensor("wvs", [2, 2, 128, 8 * 512], BF16, kind="ExternalInput").ap()
    # out-proj: [128p(hd), fc, hc, 128f]
    wps = nc.dram_tensor("wps", [128, CC, NH, 128], BF16, kind="ExternalInput").ap()
    cs2 = nc.dram_tensor("cs2", [128, T], BF16, kind="ExternalInput").ap()
    sn2 = nc.dram_tensor("sn2", [128, T], BF16, kind="ExternalInput").ap()
    # f-major partial output [2048 f, 2048 t]
    out = nc.dram_tensor("out", [D, T], BF16, kind="ExternalOutput").ap()

    with tile.TileContext(nc) as tc, ExitStack() as octx:
        # gpsimd ucode library with partition_all_reduce (loads at t=0,
        # first use is ~60 us in)
        nc.gpsimd.load_library(library_config.attn)

        cs_pool = octx.enter_context(tc.tile_pool(name="cs", bufs=1))
        csk = cs_pool.tile([128, T], BF16, tag="csk")
        snk = cs_pool.tile([128, T], BF16, tag="snk")
        jw = cs_pool.tile([128, 512], BF16, tag="jw")
        dexp = cs_pool.tile([1, 16], BF16, tag="dexp")

        oT_pool = octx.enter_context(tc.tile_pool(name="oT", bufs=1))
        oT = oT_pool.tile([128, NH, T], BF16, tag="oT")

        with ExitStack() as p1:
            wqk_pool = p1.enter_context(tc.tile_pool(name="wqk", bufs=4))
            wv_pool = p1.enter_context(tc.tile_pool(name="wv", bufs=2))
            vg_pool = p1.enter_context(tc.tile_pool(name="vg", bufs=1))
            q_pool = p1.enter_context(tc.tile_pool(name="q", bufs=2))
            k_pool = p1.enter_context(tc.tile_pool(name="k", bufs=2))
            rp_pool = p1.enter_context(tc.tile_pool(name="rp", bufs=2))
            eT_pool = p1.enter_context(tc.tile_pool(name="eT", bufs=8))
            es_pool = p1.enter_context(tc.tile_pool(name="es", bufs=2))
            eb_pool = p1.enter_context(tc.tile_pool(name="eb", bufs=2))
            rb_pool = p1.enter_context(tc.tile_pool(name="rb", bufs=2))
            ob_pool = p1.enter_context(tc.tile_pool(name="ob", bufs=4))
            wpf_pool = p1.enter_context(tc.tile_pool(name="wpf", bufs=4))
            # 8 PSUM banks: qkv/out-proj units share one pool (producers are
            # done before the out-proj starts); ps_o double-buffered so the
            # slow gpsimd norm chain reads bank A while qt+1 accumulates in
            # bank B (chain gets ~28 us of slack, PE never waits on it)
            ps_qkv = p1.enter_context(tc.tile_pool(name="psqkv", bufs=2, space="PSUM"))
            ps_s = p1.enter_context(tc.tile_pool(name="pss", bufs=4, space="PSUM"))
            ps_o = p1.enter_context(tc.tile_pool(name="pso", bufs=2, space="PSUM"))

            x_pool = p1.enter_context(tc.tile_pool(name="x", bufs=1))

            # x resident: 16 [128, 2048] tiles.  One dma_start per cc tile —
            # the sync engine issues DMAs serially at ~650 ns each, so fewer,
            # bigger issues beat fine-grained blocks (v4 measured 57 GB/s
            # effective with 64 block DMAs: issue-bound, queues 7% busy).
            xs = [None] * CC

            # per-core state holders
            q_sb = [None] * NH
            k_sb = [None] * NH
            v_sb = [None] * 2   # per group
            v_w = [[None, None], [None, None]]
            w_hold = {}
            sf_hold = {}
            wp_slabs = {}

            def load_wp_slab(fc):
                w_ = wpf_pool.tile([128, NH, 128], BF16, tag="wpf",
                                   name=f"wpf{fc}")
                nc.sync.dma_start(w_[:], wps[:, fc])
                wp_slabs[fc] = w_

            def load_vw(g, half):
                w_ = wv_pool.tile([128, 8 * 512], BF16, tag="wv")
                nc.sync.dma_start(w_[:], wvs[g, half])
                v_w[g][half] = w_

            def ensure_wqk(kind, h):
                if (kind, h) not in w_hold:
                    wsl = wqk_pool.tile([128, CC, 128], BF16, tag="wqk")
                    nc.sync.dma_start(wsl[:], (wqs if kind == "q" else wks)[h])
                    w_hold[(kind, h)] = wsl
                return w_hold[(kind, h)]

            def emit_proj_half(kind, h, tpair, half):
                """One 16-matmul unit: 512 tokens of a Q/K projection.  After
                the second half of a token-pair, the RoPE chain is emitted."""
                wsl = ensure_wqk(kind, h)
                toff = tpair * 1024
                if half == 0:
                    sf_hold[kind] = rp_pool.tile([128, 1024], BF16, tag="sf",
                                                 name=f"sf_{kind}{h}_{tpair}")
                sf = sf_hold[kind]
                ps = ps_qkv.tile([128, 512], F32, tag="psqkv")
                for cc in range(CC):
                    nc.tensor.matmul(
                        ps[:], wsl[:, cc, :],
                        xs[cc][:, toff + half * 512: toff + (half + 1) * 512],
                        start=(cc == 0), stop=(cc == CC - 1))
                # scalar copy: keeps the psum->sbuf copy out of the DVE FIFO
                # where it sat behind RoPE ops and gated the unit 2 later
                nc.scalar.copy(sf[:, ts(half, 512)], ps[:])
                if half == 0:
                    return
                sw = rp_pool.tile([128, 1024], BF16, tag="sw")
                nc.sync.dma_start(sw[0:64, :], sf[64:128, :])
                nc.sync.dma_start(sw[64:128, :], sf[0:64, :])
                nc.vector.tensor_mul(sf[:], sf[:], csk[:, toff:toff + 1024])
                nc.vector.tensor_mul(sw[:], sw[:], snk[:, toff:toff + 1024])
                dst = q_sb[h] if kind == "q" else k_sb[h]
                nc.vector.tensor_add(dst[:, toff:toff + 1024], sf[:], sw[:])

            def producer_units(hn):
                """Generator of head-hn QKV producer units (8 per head)."""
                q_sb[hn] = q_pool.tile([128, T], BF16, tag="q", name=f"qh{hn}")
                k_sb[hn] = k_pool.tile([128, T], BF16, tag="k", name=f"kh{hn}")
                for kind in ("q", "k"):
                    for tpair in range(2):
                        for half in range(2):
                            yield (kind, hn, tpair, half)

            def emit_v_chunk(g, tch_pair):
                """V for head-group g, two token chunks (2*128 tokens)."""
                for u in range(2):
                    tch = tch_pair * 2 + u
                    ps = ps_qkv.tile([128, 512], F32, tag="psqkv")
                    for cc in range(CC):
                        wv_ap = v_w[g][cc // 8][:, (cc % 8) * 512:(cc % 8 + 1) * 512]
                        nc.tensor.matmul(
                            ps[:], xs[cc][:, ts(tch, 128)], wv_ap,
                            start=(cc == 0), stop=(cc == CC - 1))
                    if tch % 2 == 0:
                        nc.scalar.copy(v_sb[g][:, tch, :], ps[:])
                    else:
                        nc.vector.tensor_copy(v_sb[g][:, tch, :], ps[:])

            op_state = {"alt": 0}

            def emit_op_unit(fc, tq, eng):
                """Partial out-proj: one [128f x 512t] tile, contraction over
                this core's 8 heads."""
                wp = wp_slabs[fc]
                ps = ps_qkv.tile([128, 512], F32, tag="psqkv")
                for hc in range(NH):
                    nc.tensor.matmul(ps[:], wp[:, hc, :],
                                     oT[:, hc, ts(tq, 512)],
                                     start=(hc == 0), stop=(hc == NH - 1))
                oe = ob_pool.tile([128, 512], BF16, tag="ob")
                if eng == "v":
                    nc.vector.tensor_copy(oe[:], ps[:])
                else:
                    nc.scalar.copy(oe[:], ps[:])
                nc.sync.dma_start(out[ts(fc, 128), ts(tq, 512)], oe[:])

            def emit_op_unit_alt(fc, tq):
                op_state["alt"] ^= 1
                emit_op_unit(fc, tq, "s" if op_state["alt"] else "v")

            # ---------------- bootstrap ----------------
            # HAM warm-up: a few junk matmuls on a memset tile bridge the
            # gap until the first x tile lands (~7 us)
            nc.vector.memset(jw[:], 0.0)
            jp = ps_qkv.tile([128, 512], F32, tag="psqkv", name="junk")
            for i in range(5):
                nc.tensor.matmul(jp[:], jw[:, 0:128], jw[:],
                                 start=True, stop=True)

            # Each issuing queue (sync=SP, scalar=Activation) serializes its
            # transfers at ~170 GB/s, so: split the 1 MB wv tiles in halves,
            # alternate x tiles between the queues in cc (=consumption)
            # order, and push weights not needed until later down the queue.
            for cc in range(CC):
                t_ = x_pool.tile([128, T], BF16, tag=f"x{cc}", name=f"x{cc}")
                xs[cc] = t_
            wv00 = wv_pool.tile([128, 8 * 512], BF16, tag="wv", name="wv00")
            v_w[0][0] = wv00
            wv01 = wv_pool.tile([128, 8 * 512], BF16, tag="wv", name="wv01")
            v_w[0][1] = wv01
            nc.sync.dma_start(wv00[:, 0:2048], wvs[0, 0, :, 0:2048])
            nc.scalar.dma_start(wv00[:, 2048:4096], wvs[0, 0, :, 2048:4096])
            nc.sync.dma_start(xs[0][:], xT[0])
            nc.scalar.dma_start(xs[1][:], xT[1])
            for cc in range(2, CC):
                eng = nc.sync if cc % 2 == 0 else nc.scalar
                eng.dma_start(xs[cc][:], xT[cc])
                if cc == 4:
                    # wv01 halves: needed from wave-A cc8 (~+14 us)
                    nc.sync.dma_start(wv01[:, 0:2048], wvs[0, 1, :, 0:2048])
                    nc.scalar.dma_start(wv01[:, 2048:4096],
                                        wvs[0, 1, :, 2048:4096])
            ensure_wqk("q", 0)
            ensure_wqk("k", 0)
            nc.scalar.dma_start(csk[:], cs2[:])
            nc.scalar.dma_start(snk[:], sn2[:])
            # dummy exp after the scalar-queue issues: walrus puts the
            # ~2.7 us act table load here, off the critical path
            nc.scalar.activation(dexp[:], csk[0:1, 0:16], EXP, scale=1.0)

            # cc-outer "wave" emission: each issuing queue serializes its
            # DMA transfers (~170 GB/s/queue), so x tiles arrive one-by-one
            # over ~25 us.  8 PSUM accumulators walked cc-outer consume each
            # xs[cc] for ~1.7 us the moment it lands instead of stalling a
            # whole 16-MM chain on the last cc.
            v_sb[0] = vg_pool.tile([128, KC, 512], BF16, tag="vg", name="vg0")
            q_sb[0] = q_pool.tile([128, T], BF16, tag="q", name="qh0")
            k_sb[0] = k_pool.tile([128, T], BF16, tag="k", name="kh0")

            def wave_psums(nm):
                return ([ps_qkv.tile([128, 512], F32, tag="psqkv",
                                     name=f"{nm}{i}") for i in range(2)]
                        + [ps_s.tile([128, 512], F32, tag="pss",
                                     name=f"{nm}{i + 2}") for i in range(4)]
                        + [ps_o.tile([128, 512], F32, tag="pso",
                                     name=f"{nm}{i + 6}") for i in range(2)])

            def v_wave(g, base):
                wt = wave_psums(f"wv{g}_{base}_")
                for cc in range(CC):
                    wv_ap = v_w[g][cc // 8][:, (cc % 8) * 512:(cc % 8 + 1) * 512]
                    for i in range(8):
                        nc.tensor.matmul(
                            wt[i][:], xs[cc][:, ts(base + i, 128)], wv_ap,
                            start=(cc == 0), stop=(cc == CC - 1))
                for i in range(8):
                    if i % 2 == 0:
                        nc.scalar.copy(v_sb[g][:, base + i, :], wt[i][:])
                    else:
                        nc.vector.tensor_copy(v_sb[g][:, base + i, :], wt[i][:])

            def qk_wave(h):
                units = [(kind, tpair, half) for kind in ("q", "k")
                         for tpair in range(2) for half in range(2)]
                wt = wave_psums(f"wqk{h}_")
                for cc in range(CC):
                    for i, (kind, tpair, half) in enumerate(units):
                        toff = tpair * 1024 + half * 512
                        nc.tensor.matmul(
                            wt[i][:], w_hold[(kind, h)][:, cc, :],
                            xs[cc][:, toff:toff + 512],
                            start=(cc == 0), stop=(cc == CC - 1))
                for kind in ("q", "k"):
                    for tpair in range(2):
                        i0 = (0 if kind == "q" else 4) + tpair * 2
                        toff = tpair * 1024
                        sf = rp_pool.tile([128, 1024], BF16, tag="sf",
                                          name=f"sfw_{kind}{tpair}")
                        nc.scalar.copy(sf[:, 0:512], wt[i0][:])
                        nc.scalar.copy(sf[:, 512:1024], wt[i0 + 1][:])
                        sw = rp_pool.tile([128, 1024], BF16, tag="sw")
                        nc.sync.dma_start(sw[0:64, :], sf[64:128, :])
                        nc.sync.dma_start(sw[64:128, :], sf[0:64, :])
                        nc.vector.tensor_mul(sf[:], sf[:], csk[:, toff:toff + 1024])
                        nc.vector.tensor_mul(sw[:], sw[:], snk[:, toff:toff + 1024])
                        dst = q_sb[h] if kind == "q" else k_sb[h]
                        nc.vector.tensor_add(dst[:, toff:toff + 1024], sf[:], sw[:])

            v_wave(0, 0)
            qk_wave(0)
            v_wave(0, 8)
            # h1 slabs (fresh wqk slots, no WAR wait) then the g1 V weights,
            # whose slot-reuse WAR wait blocks the sync queue until the g0
            # V matmuls have all read their slabs — nothing urgent behind.
            ensure_wqk("q", 1)
            ensure_wqk("k", 1)
            load_vw(1, 0)
            load_vw(1, 1)

            # Deferred softmax normalization: the gpsimd all-reduce takes
            # ~3.6 us, and the DVE queue is strict FIFO — an immediately-
            # emitted reciprocal would block the next qt's esum adds behind
            # the all-reduce (measured 4.9 us PE gap per qt).  Defer the
            # recip+normalize to kc==10 of the NEXT qt, when the all-reduce
            # result is long ready.
            pending_norms = []

            def emit_norm(h_, qt_, o_ps_t, eb_t):
                # approx recip: ~0.7 us vs 3.35 us for nc.vector.reciprocal,
                # ~51 ULP f32 — far better than the bf16 denom it replaces
                rb = rb_pool.tile([128, 512], F32, tag="rb")
                nc.vector.reciprocal_approx_fast(out=rb[:], in_=eb_t[:])
                with nc.allow_low_precision(reason="bf16 attn out"):
                    nc.vector.scalar_tensor_tensor(
                        oT[:, h_, ts(qt_, 512)], o_ps_t[:], 1.0, rb[:],
                        MULT, MULT)

            def pop_norm(lag=0):
                if len(pending_norms) > lag:
                    emit_norm(*pending_norms.pop(0))

            # ---------------- main loop: SDPA per head ----------------
            for h in range(NH):
                g, j = h // 4, h % 4
                prod = producer_units(h + 1) if h + 1 < NH else iter(())
                if h + 1 < NH:
                    # prefetch next head's Q/K slabs so the kc==1 producer
                    # unit never waits on the ~1.3 us/DMA sync issue queue
                    ensure_wqk("q", h + 1)
                    ensure_wqk("k", h + 1)
                for qt in range(4):
                    qsl = q_sb[h][:, ts(qt, 512)]
                    esum = es_pool.tile([128, 512], BF16, tag="es")
                    o_ps = ps_o.tile([128, 512], F32, tag="pso")
                    eTs = [None] * KC

                    def pv(kc):
                        nc.tensor.matmul(
                            o_ps[:], v_sb[g][:, kc, ts(j, 128)], eTs[kc][:],
                            start=(kc == 0), stop=(kc == KC - 1))

                    for kc in range(KC):
                        s_ps = ps_s.tile([128, 512], F32, tag="pss")
                        nc.tensor.matmul(s_ps[:], k_sb[h][:, ts(kc, 128)], qsl,
                                         start=True, stop=True)
                        eT = eT_pool.tile([128, 512], BF16, tag="eT")
                        nc.scalar.activation(eT[:], s_ps[:], EXP, scale=SCALE)
                        eTs[kc] = eT
                        # bf16 chunk-sum (magnitude ~30; the 2048-wide key
                        # reduction happens exactly in f32 on gpsimd below)
                        with nc.allow_low_precision(reason="bf16 chunk sum"):
                            if kc == 0:
                                nc.vector.tensor_copy(esum[:], eT[:])
                            else:
                                nc.vector.tensor_add(esum[:], esum[:], eT[:])
                        if kc >= 2:
                            pv(kc - 2)
                        if kc == 12:
                            pop_norm()
                        if h < NH - 1 and kc in (1, 7):
                            # slots early in the qt so the last unit's RoPE
                            # chain lands before the next head's first scores
                            unit = next(prod, None)
                            if unit is not None:
                                emit_proj_half(*unit)
                        if h == NH - 1:
                            if qt == 0 and kc == 1:
                                load_wp_slab(0)
                            elif qt == 0 and kc == 5:
                                load_wp_slab(1)
                            elif (qt, kc) in OP_SLOTS:
                                fc_ = OP_SLOTS[(qt, kc)]
                                emit_op_unit_alt(fc_, 0)
                                if fc_ + 2 < 10:
                                    load_wp_slab(fc_ + 2)
                            elif qt in (0, 1) and kc in (4, 6, 8, 10, 14):
                                # no fillable work exists yet in h7's first
                                # two qts (scalar-bound, ~2.8 us PE deficit
                                # per qt); burn junk MMs so PE idle gaps
                                # stay under the ~3.4 us HAM window
                                jp_ = ps_qkv.tile([128, 512], F32,
                                                  tag="psqkv",
                                                  name=f"junk7_{qt}_{kc}")
                                nc.tensor.matmul(jp_[:], jw[:, 0:128], jw[:],
                                                 start=True, stop=True)
                    pv(KC - 2)
                    pv(KC - 1)
                    # softmax denominator: f32 partition all-reduce on gpsimd
                    # (every partition gets the 512 per-token sums); the
                    # recip + (o * recip) normalize is deferred (see above).
                    eb = eb_pool.tile([128, 512], F32, tag="eb")
                    nc.gpsimd.partition_all_reduce(
                        eb[:], esum[:], 128, bass_isa.ReduceOp.add)
                    pending_norms.append((h, qt, o_ps, eb))

                # V(g1) block between head 3 and head 4: v_sb[0] reads are
                # all emitted by now, so the single vg buffer can recycle.
                if h == 3:
                    v_sb[1] = vg_pool.tile([128, KC, 512], BF16, tag="vg",
                                           name="vg1")
                    for tp in range(8):
                        emit_v_chunk(1, tp)

            # ------------- rest of the output projection -------------
            # tq0 tail (slabs 10..15 + second-round 0,1 prefetched 2 ahead);
            # the last pending norm (7,qt3) flushes once its all-reduce has
            # had ~5 us to finish
            for fc in range(8, CC):
                if fc + 2 < CC:
                    load_wp_slab(fc + 2)
                else:
                    load_wp_slab(fc + 2 - CC)
                emit_op_unit_alt(fc, 0)
                if fc == 10:
                    while pending_norms:
                        emit_norm(*pending_norms.pop(0))
            # tq 1..3 fc-major: one slab load per fc serves three units
            for fc in range(CC):
                for i, tq in enumerate((1, 2, 3)):
                    if i == 0 and fc + 2 < CC:
                        load_wp_slab(fc + 2)
                    emit_op_unit_alt(fc, tq)

    # populate .instr bytes for extended-inst InstISA subclasses
    # (partition_all_reduce) — raw Bass doesn't run this pass and the NEFF
    # compiler rejects empty .instr with "ISA wrong length"
    mybir.codegen_inst_isa_subclasses(nc)
    _split_multi_waits(nc)
    return nc


# ---------------------------------------------------------------------------
# host-side prep / assembly
# ---------------------------------------------------------------------------


def _to_bf16(a):
    return np.ascontiguousarray(a.astype(ml_dtypes.bfloat16))


def prep_inputs(x, w_attn, w_proj):
    x = np.asarray(x, dtype=np.float32)
    w_attn = np.asarray(w_attn, dtype=np.float32)
    w_proj = np.asarray(w_proj, dtype=np.float32)

    perm = np.concatenate([np.arange(0, HD, 2), np.arange(1, HD, 2)])

    inv = 1.0 / (10000.0 ** (np.arange(0, HD, 2, dtype=np.float64) / HD))
    fr = np.outer(np.arange(T, dtype=np.float64), inv)
    cos = np.cos(fr).T
    sin = np.sin(fr).T
    cs2 = _to_bf16(np.concatenate([cos, cos], 0))
    sn2 = _to_bf16(np.concatenate([-sin, sin], 0))

    # per head-half weight slabs (shared across batches)
    half_slabs = []
    for hh in range(2):
        heads = range(hh * NH, (hh + 1) * NH)
        # wq/wk: [NH, 128p(c within cc), CC*128f] with rope perm on f
        wq_sl = np.empty((NH, 128, CC * 128), dtype=np.float32)
        wk_sl = np.empty((NH, 128, CC * 128), dtype=np.float32)
        for jj, h in enumerate(heads):
            wq_h = w_attn[h * HD:(h + 1) * HD][perm, :]        # [128f, 2048c]
            wk_h = w_attn[D + h * HD:D + (h + 1) * HD][perm, :]
            # slab[p, cc, f] = w[f, cc*128+p]
            wq_sl[jj] = wq_h.T.reshape(CC, 128, 128).transpose(1, 0, 2).reshape(128, -1)
            wk_sl[jj] = wk_h.T.reshape(CC, 128, 128).transpose(1, 0, 2).reshape(128, -1)
        # wv: [2 groups, 2 halves, 128p, 8cc*(4h*128)]
        wv_sl = np.empty((2, 2, 128, 8 * 512), dtype=np.float32)
        for g in range(2):
            hv = w_attn[2 * D + (hh * NH + g * 4) * HD:
                        2 * D + (hh * NH + (g + 1) * 4) * HD]  # [512f, 2048c]
            # [cc, p, f] -> [half, 128p, 8cc, 512f]
            arr = hv.T.reshape(CC, 128, 512)
            for half in range(2):
                wv_sl[g, half] = (arr[half * 8:(half + 1) * 8]
                                  .transpose(1, 0, 2).reshape(128, -1))
        # wp: [128p(hd within hc), fc, hc, 128f]
        #   value = w_proj[fc*128+f, hh*1024 + hc*128 + p]
        wp_cols = w_proj[:, hh * NH * HD:(hh + 1) * NH * HD]  # [2048f, 1024hd]
        wp_sl = (wp_cols.T.reshape(NH, 128, CC, 128)
                 .transpose(1, 2, 0, 3))                       # [128p, fc, hc, f]
        half_slabs.append((_to_bf16(wq_sl), _to_bf16(wk_sl), _to_bf16(wv_sl),
                           _to_bf16(np.ascontiguousarray(wp_sl))))

    xTs = []
    for b in range(B):
        xT = x[b].T.reshape(CC, 128, T)
        xTs.append(_to_bf16(xT))

    in_maps = []
    for i in range(N_CORES):
        b, hh = i // 2, i % 2
        wq_sl, wk_sl, wv_sl, wp_sl = half_slabs[hh]
        in_maps.append({
            "xT": xTs[b],
            "wqs": wq_sl, "wks": wk_sl, "wvs": wv_sl, "wps": wp_sl,
            "cs2": cs2, "sn2": sn2,
        })
    return in_maps


def assemble(results):
    out = np.empty((B, T, D), dtype=np.float32)
    for b in range(B):
        p0 = results[2 * b]["out"].astype(np.float32)
        p1 = results[2 * b + 1]["out"].astype(np.float32)
        out[b] = (p0 + p1).T
    return out


_nc_cache = None


def _get_nc():
    global _nc_cache
    if _nc_cache is None:
        _nc_cache = build_nc()
    return _nc_cache


def kernel(x, w_attn, w_proj):
    from concourse.bass_utils import run_bass_kernel_spmd
    nc = _get_nc()
    in_maps = prep_inputs(x, w_attn, w_proj)
    res = run_bass_kernel_spmd(nc, in_maps, list(range(N_CORES)))
    return assemble(res.results)


def run_profiled(x, w_attn, w_proj, trace_cores=None):
    """Like kernel() but with NTFF profiling; returns BassKernelResults."""
    from concourse.bass_utils import run_bass_kernel_spmd
    import sys as _sys, types as _types
    try:
        import antenv
        if "antenv.axon_hooks" not in _sys.modules:
            mod = _types.ModuleType("antenv.axon_hooks")
            _h = [None]
            mod.set_axon_ntff_profile_hook = lambda h: _h.__setitem__(0, h)
            mod.get_axon_ntff_profile_hook = lambda: _h[0]
            _sys.modules["antenv.axon_hooks"] = mod
            antenv.axon_hooks = mod
            from trn_agent_boot.trn_boot import _ntff_profile_via_ctypes
            mod.set_axon_ntff_profile_hook(
                _ntff_profile_via_ctypes('/opt/axon/libaxon_pjrt.so'))
    except Exception as e:  # profiling is best-effort
        print("profile hook setup failed:", e)
    nc = _get_nc()
    in_maps = prep_inputs(x, w_attn, w_proj)
    return run_bass_kernel_spmd(
        nc, in_maps, list(range(N_CORES)), trace=True,
        trace_cores=trace_cores if trace_cores is not None else [0])
